# revision 40
# baseline (speedup 1.0000x reference)
"""Trainium2 Bass kernel for GCN(x2) + MHA + mean + FC, sharded over 8 NeuronCores.

Sharding: 1D row partition of the 4096 nodes (512 rows/core). Each core holds
the column slice adj_hat[:, r*512:(r+1)*512] of the symmetric A+I (by symmetry
equal to its row block transposed), all of x, and replicated weights.
Cross-core exchanges (on-device AllGather): degree vector, GCN1 output, K/V.

Key layout choices (v2):
  - All big operands are host-packed into "tile-major" [128, C] DRAM arrays so
    every load is ONE large contiguous-per-partition DMA (the HWDGE fixed
    overhead is serialized across DMAs, so DMA count dominates; the baseline's
    260 DMAs are cut to ~40).
  - Activations stay in [feature, node] / tile-packed layouts so no transposes
    are needed; matmuls run in bf16 with fp32 PSUM accumulation.
  - Constant-bias folding on host: b2 is folded into the Q/K/V biases, the V
    bias and the out_proj bias are folded together, so the device adds biases
    only where they vary per partition.
  - Softmax: exp on ACT in [128,1024] batches from 2-bank PSUM score tiles;
    denominator via a bf16 running sum on DVE (2x perf mode) + a tiny
    ones-matmul partition reduction; ctx stays in PSUM until the final
    normalize + query-sum reduce produces the per-head z.
  - PSUM rule learned on HW: start_tensor_calc clears has_written for the
    whole bank, so accumulation groups never interleave within a bank.
Host does only slicing/packing/dtype casts (and constant-parameter folding)
plus an 8-way sum of [2]-vector partials at the end.
"""
import sys
sys.path.insert(0, "/opt/trn_rl_repo")
import numpy as np
import ml_dtypes

N = 4096
NC_ = 8
R = N // NC_          # 512 rows per core
KB = N // 128         # 32 node chunks
F_IN = 128
G1 = 128
G2 = 512
HEADS = 4
HD = G2 // HEADS      # 128
ET = G2 // 128        # 4 tiles of the 512-dim embedding

# wp (bf16) column layout
WP_W1 = 0
WP_W2 = WP_W1 + F_IN          # 128
WP_WIN = WP_W2 + G2           # 640
WP_COLS = WP_WIN + ET * 3 * G2  # 640 + 6144 = 6784

# bp (fp32) column layout
BP_BQ = 0                      # [128,4] per-head q bias (b2 folded in)
BP_BK = 4                      # [128,4]
BP_B1 = 8                      # [128,128] b1 replicated across partitions
BP_BO = 136                    # [128,4] (bo + bv_eff@wo)/8 col-layout
BP_WO = 140                    # [128, 2048] out_proj 4 part-tiles
BP_FCW = BP_WO + ET * G2       # 2188: [128, 8] fc_w 4 part-tiles
BP_FCB = BP_FCW + 8            # 2196: [1,2] fc_b/8 (row 0)
BP_COLS = BP_FCB + 2           # 2198

_cache = {}


def _build(sim1=False):
    from concourse import bass, bacc, tile, mybir

    f32 = mybir.dt.float32
    bf16 = mybir.dt.bfloat16
    AF = mybir.ActivationFunctionType
    ALU = mybir.AluOpType
    AX = mybir.AxisListType

    nc = bacc.Bacc("TRN2", target_bir_lowering=False, debug=False,
                   num_devices=1 if sim1 else NC_)

    adj_d = nc.dram_tensor("adjp", [128, KB * R], bf16, kind="ExternalInput")
    x_d = nc.dram_tensor("xp", [128, KB * F_IN], bf16, kind="ExternalInput")
    wp_d = nc.dram_tensor("wp", [128, WP_COLS], bf16, kind="ExternalInput")
    bp_d = nc.dram_tensor("bp", [128, BP_COLS], f32, kind="ExternalInput")
    out_d = nc.dram_tensor("outp", [1, 2], f32, kind="ExternalOutput")

    RG = [list(range(NC_))]
    inv_sqrt_hd = 1.0 / float(np.sqrt(HD))

    with tile.TileContext(nc) as tc:
        with tc.tile_pool(name="psbig", bufs=2, space="PSUM") as psbig, \
             tc.tile_pool(name="psqkv", bufs=1, space="PSUM") as psqkv, \
             tc.tile_pool(name="psctx", bufs=1, space="PSUM") as psctx, \
             tc.tile_pool(name="pssm", bufs=1, space="PSUM") as pssm, \
             tc.tile_pool(name="big", bufs=1) as bigp, \
             tc.tile_pool(name="kv", bufs=2) as kvp, \
             tc.tile_pool(name="pt", bufs=3) as ptp, \
             tc.tile_pool(name="sm", bufs=2) as smp, \
             tc.tile_pool(name="smq", bufs=2) as smqp, \
             tc.tile_pool(name="dram", bufs=1, space="DRAM") as drp:

            # ---------------- constants / params ----------------
            ones_b = bigp.tile([128, 1], bf16, tag="ones_b")
            nc.vector.memset(ones_b[:], 1.0)

            # All big loads go on the SP (sync) queue in priority order —
            # SEQs issue in order, so this fixes the DMA_ENGINES schedule:
            # adjacency (gates degree) > x > [degree-chain DMAs] > biases >
            # weights (first needed by the W1 matmul, well after GCN1).
            adj_sb = bigp.tile([128, KB * R], bf16, tag="adj")
            NLOAD = 8
            CW = KB * R // NLOAD
            for k in range(NLOAD):
                nc.sync.dma_start(adj_sb[:, k * CW:(k + 1) * CW],
                                  adj_d[:, k * CW:(k + 1) * CW])
            xp_sb = bigp.tile([128, KB * F_IN], bf16, tag="xp")
            nc.sync.dma_start(xp_sb[:], x_d[:, :])
            wp_sb = bigp.tile([128, WP_COLS], bf16, tag="wp")
            bp_sb = bigp.tile([128, BP_COLS], f32, tag="bp")

            def w1_ap():
                return wp_sb[:, WP_W1:WP_W1 + G1]

            def w2_ap(et):
                return wp_sb[:, WP_W2 + et * 128:WP_W2 + (et + 1) * 128]

            def win_ap(c, kind, h):
                off = WP_WIN + c * 3 * G2 + kind * G2 + h * HD
                return wp_sb[:, off:off + HD]

            def adj_ap(kb):
                return adj_sb[:, kb * R:(kb + 1) * R]

            # ---------------- degree ----------------
            ps_deg = pssm.tile([1, R], f32, tag="sm")
            for kb in range(KB):
                nc.tensor.matmul(ps_deg[:], ones_b[:], adj_ap(kb),
                                 start=(kb == 0), stop=(kb == KB - 1))
            sq = smp.tile([1, R], f32, tag="sq")
            nc.scalar.activation(sq[:], ps_deg[:], AF.Sqrt)
            dloc = bigp.tile([1, R], f32, tag="dloc")
            nc.vector.reciprocal(dloc[:], sq[:])
            # pull the Exp ACT-table load off the first-attention critical
            # path: a dummy exp here makes bacc place the table switch now,
            # while the Activation engine is otherwise idle.
            expwarm = smp.tile([1, 1], f32, tag="expwarm")
            nc.scalar.activation(expwarm[:], sq[0:1, 0:1], AF.Exp)

            # AG1: gather 1/sqrt(deg) across cores
            dg_in = drp.tile([1, R], f32, tag="dgin")
            dg_out = nc.dram_tensor("dg_out", [NC_, R], f32, kind="Internal",
                                    addr_space="Shared")
            nc.sync.dma_start(dg_in[:], dloc[:])
            if sim1:
                nc.sync.dma_start(dg_out[0:1, :], dg_in[:])
            else:
                nc.gpsimd.collective_compute(
                    "AllGather", ALU.bypass, replica_groups=RG,
                    ins=[dg_in.opt()], outs=[dg_out.ap()])
            # d for all nodes -> dcol [128, KB]: per-column reads (the fused
            # small-stride gather pattern faults the DMA engines on HW)
            dcol = bigp.tile([128, KB], f32, tag="dcol")
            for kb in range(KB):
                rr, jb = kb // ET, kb % ET
                nc.scalar.dma_start(dcol[:, kb:kb + 1],
                                    dg_out[rr:rr + 1, jb * 128:(jb + 1) * 128])
            # params can load behind the degree chain (SP queue order)
            nc.sync.dma_start(bp_sb[:], bp_d[:, :])
            nc.sync.dma_start(wp_sb[:], wp_d[:, :])
            # own-column scale broadcast [128, R]
            dbc = bigp.tile([128, R], f32, tag="dbc")
            nc.gpsimd.partition_broadcast(dbc[:], dloc[:])

            # ---------------- x scale + GCN1 ----------------
            xs_sb = bigp.tile([128, KB * F_IN], bf16, tag="xs")
            for kb in range(KB):
                nc.vector.tensor_scalar_mul(
                    xs_sb[:, kb * F_IN:(kb + 1) * F_IN],
                    xp_sb[:, kb * F_IN:(kb + 1) * F_IN],
                    dcol[:, kb:kb + 1])
            ps_s1 = psqkv.tile([128, R], f32, tag="qkv")
            for kb in range(KB):
                nc.tensor.matmul(ps_s1[:], xs_sb[:, kb * F_IN:(kb + 1) * F_IN],
                                 adj_ap(kb), start=(kb == 0), stop=(kb == KB - 1))
            s1t = bigp.tile([128, R], bf16, tag="s1t")
            nc.vector.tensor_mul(s1t[:], ps_s1[:], dbc[:])
            # x1 = relu(s1.T @ W1 + b1), tile-packed [128, 4*128]
            psx = psqkv.tile([128, R], f32, tag="qkv")
            for mt in range(ET):
                nc.tensor.matmul(psx[:, mt * 128:(mt + 1) * 128],
                                 s1t[:, mt * 128:(mt + 1) * 128],
                                 w1_ap(), start=True, stop=True,
                                 skip_group_check=True)
            x1sb = bigp.tile([128, R], bf16, tag="x1sb")
            for mt in range(ET):
                tmp = smp.tile([128, G1], f32, tag="x1tmp")
                nc.vector.tensor_add(tmp[:], psx[:, mt * 128:(mt + 1) * 128],
                                     bp_sb[:, BP_B1:BP_B1 + G1])
                nc.scalar.activation(x1sb[:, mt * 128:(mt + 1) * 128], tmp[:],
                                     AF.Relu)

            # AG2: gather x1 (tile-packed per rank), pipelined in two column
            # halves (half = node-chunks {0,1} / {2,3} of every rank) so GCN2
            # can start on the first half while the second is still in flight.
            HC = R // 2
            x1g = bigp.tile([128, KB * G1], bf16, tag="x1g")
            for half in range(2):
                cs = half * HC
                x1_in = drp.tile([128, HC], bf16, tag=f"x1in{half}")
                x1_out = nc.dram_tensor(f"x1_out{half}", [NC_ * 128, HC], bf16,
                                        kind="Internal", addr_space="Shared")
                nc.scalar.dma_start(x1_in[:], x1sb[:, cs:cs + HC])
                if sim1:
                    nc.scalar.dma_start(x1_out[0:128, :], x1_in[:])
                else:
                    nc.gpsimd.collective_compute(
                        "AllGather", ALU.bypass, replica_groups=RG,
                        ins=[x1_in.opt()], outs=[x1_out.ap()])
                # per-rank 2D slice reads (proven DMA family on HW)
                for rr in range(NC_):
                    nc.sync.dma_start(
                        x1g[:, rr * R + cs:rr * R + cs + HC],
                        x1_out[rr * 128:(rr + 1) * 128, :])

            # ---------------- GCN2 ----------------
            # kb order follows AG2 half arrival: chunks {4r,4r+1} then {4r+2,4r+3}
            kb_order = [r * ET + mt for half in range(2)
                        for r in range(NC_) for mt in (half * 2, half * 2 + 1)]
            x1s = bigp.tile([128, KB * G1], bf16, tag="x1s")
            for kb in kb_order:
                nc.vector.tensor_scalar_mul(
                    x1s[:, kb * G1:(kb + 1) * G1],
                    x1g[:, kb * G1:(kb + 1) * G1],
                    dcol[:, kb:kb + 1])
            ps_s2 = psqkv.tile([128, R], f32, tag="qkv")
            for n, kb in enumerate(kb_order):
                nc.tensor.matmul(ps_s2[:], x1s[:, kb * G1:(kb + 1) * G1],
                                 adj_ap(kb), start=(n == 0), stop=(n == KB - 1))
            s2t = bigp.tile([128, R], bf16, tag="s2t")
            nc.vector.tensor_mul(s2t[:], ps_s2[:], dbc[:])
            # x2.T tiles [e-chunk 128, node 512] (b2 folded into QKV biases);
            # two 2-bank score-ring tiles so all 4 matmuls+copies pipeline.
            x2t_sb = []
            for half in range(2):
                ps2 = psbig.tile([128, 2 * R], f32, tag="sc")
                for i in range(2):
                    nc.tensor.matmul(ps2[:, i * R:(i + 1) * R],
                                     w2_ap(half * 2 + i), s2t[:],
                                     start=True, stop=True,
                                     skip_group_check=True)
                for i in range(2):
                    et = half * 2 + i
                    xt = bigp.tile([128, R], bf16, tag=f"x2_{et}")
                    if i == 0:
                        nc.vector.tensor_copy(xt[:], ps2[:, i * R:(i + 1) * R])
                    else:
                        nc.scalar.activation(xt[:], ps2[:, i * R:(i + 1) * R],
                                             AF.Identity)
                    x2t_sb.append(xt)

            # ---------------- QKV + per-head AllGather ----------------
            qt_sb = {}
            kv_out = {}
            for h in range(HEADS):
                # K^T and V share one 2-bank tile; Q reuses K's half after
                # the K bias-add has drained it.
                kvq = psqkv.tile([128, 2 * R], f32, tag="qkv")
                psk = kvq[:, 0:R]
                psv = kvq[:, R:2 * R]
                for c in range(ET):
                    nc.tensor.matmul(psk, win_ap(c, 1, h), x2t_sb[c][:],
                                     start=(c == 0), stop=(c == ET - 1),
                                     skip_group_check=True)
                kvsb = smqp.tile([128, 2 * R], bf16, tag="kvloc")
                nc.vector.tensor_scalar_add(kvsb[:, 0:R], psk,
                                            bp_sb[:, BP_BK + h:BP_BK + h + 1])
                # V tile-packed [128, 4*128] (no bias: folded into out bias)
                for mt in range(ET):
                    for c in range(ET):
                        nc.tensor.matmul(
                            psv[:, mt * HD:(mt + 1) * HD],
                            x2t_sb[c][:, mt * 128:(mt + 1) * 128],
                            win_ap(c, 2, h),
                            start=(c == 0), stop=(c == ET - 1),
                            skip_group_check=True)
                nc.vector.tensor_copy(kvsb[:, R:2 * R], psv)
                # bounce + AllGather: rank block = [KT(128) ; Vpack(128)]
                kvi = drp.tile([256, R], bf16, tag=f"kvi{h}")
                kvo = nc.dram_tensor(f"kvo{h}", [NC_ * 256, R], bf16,
                                     kind="Internal", addr_space="Shared")
                nc.scalar.dma_start(
                    kvi[:, :].rearrange("(t p) j -> p t j", t=2),
                    kvsb[:].rearrange("p (t j) -> p t j", t=2))
                if sim1:
                    nc.scalar.dma_start(kvo[0:256, :], kvi[:])
                else:
                    nc.gpsimd.collective_compute(
                        "AllGather", ALU.bypass, replica_groups=RG,
                        ins=[kvi.opt()], outs=[kvo.ap()])
                kv_out[h] = kvo
                # Q^T [d, own nodes] (overlaps the AllGather); own ring slot
                # (the ring's WAR dep serializes it behind kt/vcopy reads)
                qq = psqkv.tile([128, 2 * R], f32, tag="qkv")
                psq = qq[:, 0:R]
                for c in range(ET):
                    nc.tensor.matmul(psq, win_ap(c, 0, h), x2t_sb[c][:],
                                     start=(c == 0), stop=(c == ET - 1),
                                     skip_group_check=True)
                qt = kvp.tile([128, R], bf16, tag="qt")
                nc.vector.tensor_scalar_add(qt[:], psq,
                                            bp_sb[:, BP_BQ + h:BP_BQ + h + 1])
                qt_sb[h] = qt
                # gathered K/V loads issued here (sync queue) so earlier
                # heads' loads aren't stuck behind later heads' bounce writes
                kv3 = kvo[:, :].rearrange("(r t p) j -> t p r j", t=2, p=128)
                ktg = kvp.tile([128, NC_ * R], bf16, tag="ktg")
                nc.sync.dma_start(
                    ktg[:].rearrange("p (r j) -> p r j", r=NC_), kv3[0])
                vg = kvp.tile([128, NC_ * R], bf16, tag="vg")
                nc.sync.dma_start(
                    vg[:].rearrange("p (r j) -> p r j", r=NC_), kv3[1])
                kv_out[h] = (ktg, vg)

            # ---------------- attention per head ----------------
            # NOTE: start_tensor_calc clears has_written for the WHOLE PSUM
            # bank, so accumulation groups must not interleave within a bank
            # (a later group's start turns an earlier group's next accumulate
            # into an overwrite). u is therefore accumulated in the tail with
            # strictly sequential groups.
            zhold = []
            GRP = 2            # key chunks per score tile
            NG = KB // GRP     # 16 groups
            for h in range(HEADS):
                ktg, vg = kv_out[h]
                ps_ctx = psctx.tile([128, R], f32, tag="ctx")
                acc = smqp.tile([128, GRP * R], bf16, tag="acc")
                # software-pipelined: PV of group g is emitted alongside the
                # score matmuls of group g+2, so the in-order PE queue never
                # blocks on an exp that hasn't finished.
                SKEW = 2
                pts = {}
                for g in range(NG + SKEW):
                    if g < NG:
                        psc = psbig.tile([128, GRP * R], f32, tag="sc")
                        for i in range(GRP):
                            kc = g * GRP + i
                            nc.tensor.matmul(psc[:, i * R:(i + 1) * R],
                                             ktg[:, kc * 128:(kc + 1) * 128],
                                             qt_sb[h][:], start=True, stop=True,
                                             skip_group_check=True)
                        pt = ptp.tile([128, GRP * R], bf16, tag="pt")
                        nc.scalar.activation(pt[:], psc[:], AF.Exp,
                                             scale=inv_sqrt_hd)
                        pts[g] = pt
                        if g == 0:
                            nc.vector.tensor_copy(acc[:], pt[:])
                        else:
                            nc.vector.tensor_add(acc[:], acc[:], pt[:])
                    gp = g - SKEW
                    if gp >= 0:
                        pt = pts.pop(gp)
                        for i in range(GRP):
                            kc = gp * GRP + i
                            nc.tensor.matmul(ps_ctx[:],
                                             vg[:, kc * 128:(kc + 1) * 128],
                                             pt[:, i * R:(i + 1) * R],
                                             start=(gp == 0 and i == 0),
                                             stop=(gp == NG - 1 and i == GRP - 1),
                                             skip_group_check=True)
                # denominator: partition-reduce acc via ones matmul
                ps_den = pssm.tile([1, R], f32, tag="sm")
                for i in range(GRP):
                    nc.tensor.matmul(ps_den[:], ones_b[:],
                                     acc[:, i * R:(i + 1) * R],
                                     start=(i == 0), stop=(i == GRP - 1),
                                     skip_group_check=True)
                rden = smp.tile([1, R], f32, tag="rden")
                nc.vector.reciprocal(rden[:], ps_den[:])
                rbc = smp.tile([128, R], f32, tag="rbc")
                nc.gpsimd.partition_broadcast(rbc[:], rden[:])
                ctxn = smp.tile([128, R], f32, tag="ctxn")
                nc.vector.tensor_mul(ctxn[:], ps_ctx[:], rbc[:])
                zf = smqp.tile([128, 1], f32, tag=f"z{h}")
                nc.vector.tensor_reduce(zf[:], ctxn[:], axis=AX.X, op=ALU.add)
                zhold.append(zf)

            # ---------------- out_proj + mean + fc (partial) ----------------
            psu = psqkv.tile([128, ET], f32, tag="qkv")
            u_sb = []
            for et in range(ET):
                for h in range(HEADS):
                    wo_c = bp_sb[:, BP_WO + h * G2 + et * 128:
                                 BP_WO + h * G2 + (et + 1) * 128]
                    nc.tensor.matmul(psu[:, et:et + 1], wo_c, zhold[h][:],
                                     start=(h == 0), stop=(h == HEADS - 1),
                                     skip_group_check=True)
                ut = smp.tile([128, 1], f32, tag=f"u{et}")
                nc.vector.tensor_scalar_mul(ut[:], psu[:, et:et + 1],
                                            1.0 / float(N))
                nc.vector.tensor_add(ut[:], ut[:],
                                     bp_sb[:, BP_BO + et:BP_BO + et + 1])
                u_sb.append(ut)
            ps_fc = pssm.tile([1, 2], f32, tag="sm")
            for c in range(ET):
                nc.tensor.matmul(ps_fc[:], u_sb[c][:],
                                 bp_sb[:, BP_FCW + c * 2:BP_FCW + (c + 1) * 2],
                                 start=(c == 0), stop=(c == ET - 1))
            ores = smp.tile([1, 2], f32, tag="ores")
            nc.vector.tensor_add(ores[:], ps_fc[:],
                                 bp_sb[0:1, BP_FCB:BP_FCB + 2])
            nc.sync.dma_start(out_d[:, :], ores[:])

    nc.compile()
    return nc


def _pack_tiles(a, p=128):
    """[n*p, c] row-major -> [p, n*c] tile-packed (partition = row % p)."""
    n = a.shape[0] // p
    return np.ascontiguousarray(
        a.reshape(n, p, a.shape[1]).transpose(1, 0, 2).reshape(p, -1))


def kernel(**inputs):
    from concourse.bass_utils import run_bass_kernel_spmd

    if "nc" not in _cache:
        _cache["nc"] = _build()
    nc = _cache["nc"]

    bf = ml_dtypes.bfloat16
    adj = np.ascontiguousarray(inputs["adj_matrix"], dtype=np.float32)
    x = np.ascontiguousarray(inputs["node_features"], dtype=np.float32)
    w1 = np.asarray(inputs["W1"], np.float32)
    b1 = np.asarray(inputs["b1"], np.float32)
    w2 = np.asarray(inputs["W2"], np.float32)
    b2 = np.asarray(inputs["b2"], np.float32)
    win = np.asarray(inputs["in_proj_w"], np.float32)
    bin_ = np.asarray(inputs["in_proj_b"], np.float32)
    wo = np.asarray(inputs["out_proj_w"], np.float32)
    bo = np.asarray(inputs["out_proj_b"], np.float32)
    fcw = np.asarray(inputs["fc_w"], np.float32)
    fcb = np.asarray(inputs["fc_b"], np.float32)

    # ---- constant-parameter folding (host, exact math on fp32 weights) ----
    # x2 is only consumed by the QKV projections, so fold b2 through them.
    bq_eff = b2 @ win[:, 0:G2] + bin_[0:G2]
    bk_eff = b2 @ win[:, G2:2 * G2] + bin_[G2:2 * G2]
    bv_eff = b2 @ win[:, 2 * G2:3 * G2] + bin_[2 * G2:3 * G2]
    # V bias passes through softmax untouched (weights sum to 1), so it adds
    # bv_eff @ wo to every row of attn_out; fold into the mean+out_proj bias.
    bo_eff8 = (bo + bv_eff @ wo) / NC_

    wp = np.concatenate([
        w1, w2, _pack_tiles(win)], axis=1).astype(bf)
    bp = np.zeros((128, BP_COLS), np.float32)
    bp[:, BP_BQ:BP_BQ + 4] = bq_eff.reshape(4, 128).T
    bp[:, BP_BK:BP_BK + 4] = bk_eff.reshape(4, 128).T
    bp[:, BP_B1:BP_B1 + G1] = np.broadcast_to(b1, (128, G1))
    bp[:, BP_BO:BP_BO + 4] = bo_eff8.reshape(4, 128).T
    bp[:, BP_WO:BP_WO + ET * G2] = _pack_tiles(wo)
    bp[:, BP_FCW:BP_FCW + 8] = _pack_tiles(fcw)
    bp[0, BP_FCB:BP_FCB + 2] = fcb / NC_

    xp = _pack_tiles(x).astype(bf)
    reps = {"xp": xp, "wp": wp, "bp": bp}

    in_maps = []
    idx = np.arange(R)
    for r in range(NC_):
        cols = np.ascontiguousarray(adj[:, r * R:(r + 1) * R])
        cols[r * R + idx, idx] += 1.0   # A + I, this core's diagonal block
        in_maps.append({"adjp": _pack_tiles(cols).astype(bf), **reps})

    res = run_bass_kernel_spmd(nc, in_maps, core_ids=list(range(NC_)))
    out = np.zeros(2, dtype=np.float64)
    for r in range(NC_):
        out += res.results[r]["outp"].reshape(2).astype(np.float64)
    return out.astype(np.float32)


# revision 41
# speedup vs baseline: 1.0599x; 1.0599x over previous
"""Trainium2 Bass kernel for GCN(x2) + MHA + mean + FC, sharded over 8 NeuronCores.

Sharding: 1D row partition of the 4096 nodes (512 rows/core). Each core holds
the column slice adj_hat[:, r*512:(r+1)*512] of the symmetric A+I (by symmetry
equal to its row block transposed), all of x, and replicated weights.
Cross-core exchanges (on-device AllGather): degree vector, GCN1 output, K/V.

Key layout choices (v2):
  - All big operands are host-packed into "tile-major" [128, C] DRAM arrays so
    every load is ONE large contiguous-per-partition DMA (the HWDGE fixed
    overhead is serialized across DMAs, so DMA count dominates; the baseline's
    260 DMAs are cut to ~40).
  - Activations stay in [feature, node] / tile-packed layouts so no transposes
    are needed; matmuls run in bf16 with fp32 PSUM accumulation.
  - Constant-bias folding on host: b2 is folded into the Q/K/V biases, the V
    bias and the out_proj bias are folded together, so the device adds biases
    only where they vary per partition.
  - Softmax: exp on ACT in [128,1024] batches from 2-bank PSUM score tiles;
    denominator via a bf16 running sum on DVE (2x perf mode) + a tiny
    ones-matmul partition reduction; ctx stays in PSUM until the final
    normalize + query-sum reduce produces the per-head z.
  - PSUM rule learned on HW: start_tensor_calc clears has_written for the
    whole bank, so accumulation groups never interleave within a bank.
Host does only slicing/packing/dtype casts (and constant-parameter folding)
plus an 8-way sum of [2]-vector partials at the end.
"""
import sys
sys.path.insert(0, "/opt/trn_rl_repo")
import numpy as np
import ml_dtypes

N = 4096
NC_ = 8
R = N // NC_          # 512 rows per core
KB = N // 128         # 32 node chunks
F_IN = 128
G1 = 128
G2 = 512
HEADS = 4
HD = G2 // HEADS      # 128
ET = G2 // 128        # 4 tiles of the 512-dim embedding

# wp (bf16) column layout
WP_W1 = 0
WP_W2 = WP_W1 + F_IN          # 128
WP_WIN = WP_W2 + G2           # 640
WP_COLS = WP_WIN + ET * 3 * G2  # 640 + 6144 = 6784

# bp (fp32) column layout
BP_BQ = 0                      # [128,4] per-head q bias (b2 folded in)
BP_BK = 4                      # [128,4]
BP_B1 = 8                      # [128,128] b1 replicated across partitions
BP_BO = 136                    # [128,4] (bo + bv_eff@wo)/8 col-layout
BP_WO = 140                    # [128, 2048] out_proj 4 part-tiles
BP_FCW = BP_WO + ET * G2       # 2188: [128, 8] fc_w 4 part-tiles
BP_FCB = BP_FCW + 8            # 2196: [1,2] fc_b/8 (row 0)
BP_COLS = BP_FCB + 2           # 2198

_cache = {}


def _build(sim1=False):
    from concourse import bass, bacc, tile, mybir

    f32 = mybir.dt.float32
    bf16 = mybir.dt.bfloat16
    AF = mybir.ActivationFunctionType
    ALU = mybir.AluOpType
    AX = mybir.AxisListType

    nc = bacc.Bacc("TRN2", target_bir_lowering=False, debug=False,
                   num_devices=1 if sim1 else NC_)

    adj_d = nc.dram_tensor("adjp", [128, KB * R], bf16, kind="ExternalInput")
    x_d = nc.dram_tensor("xp", [128, KB * F_IN], bf16, kind="ExternalInput")
    wp_d = nc.dram_tensor("wp", [128, WP_COLS], bf16, kind="ExternalInput")
    bp_d = nc.dram_tensor("bp", [128, BP_COLS], f32, kind="ExternalInput")
    out_d = nc.dram_tensor("outp", [1, 2], f32, kind="ExternalOutput")

    RG = [list(range(NC_))]
    inv_sqrt_hd = 1.0 / float(np.sqrt(HD))

    with tile.TileContext(nc) as tc:
        with tc.tile_pool(name="psbig", bufs=2, space="PSUM") as psbig, \
             tc.tile_pool(name="psqkv", bufs=1, space="PSUM") as psqkv, \
             tc.tile_pool(name="psctx", bufs=1, space="PSUM") as psctx, \
             tc.tile_pool(name="pssm", bufs=1, space="PSUM") as pssm, \
             tc.tile_pool(name="big", bufs=1) as bigp, \
             tc.tile_pool(name="kv", bufs=2) as kvp, \
             tc.tile_pool(name="pt", bufs=3) as ptp, \
             tc.tile_pool(name="sm", bufs=2) as smp, \
             tc.tile_pool(name="smq", bufs=2) as smqp, \
             tc.tile_pool(name="dram", bufs=1, space="DRAM") as drp:

            # ---------------- constants / params ----------------
            ones_b = bigp.tile([128, 1], bf16, tag="ones_b")
            nc.vector.memset(ones_b[:], 1.0)

            # All big loads go on the SP (sync) queue in priority order —
            # SEQs issue in order, so this fixes the DMA_ENGINES schedule:
            # adjacency (gates degree) > x > [degree-chain DMAs] > biases >
            # weights (first needed by the W1 matmul, well after GCN1).
            adj_sb = bigp.tile([128, KB * R], bf16, tag="adj")
            NLOAD = 8
            CW = KB * R // NLOAD
            for k in range(NLOAD):
                nc.sync.dma_start(adj_sb[:, k * CW:(k + 1) * CW],
                                  adj_d[:, k * CW:(k + 1) * CW])
            xp_sb = bigp.tile([128, KB * F_IN], bf16, tag="xp")
            nc.sync.dma_start(xp_sb[:], x_d[:, :])
            wp_sb = bigp.tile([128, WP_COLS], bf16, tag="wp")
            bp_sb = bigp.tile([128, BP_COLS], f32, tag="bp")

            def w1_ap():
                return wp_sb[:, WP_W1:WP_W1 + G1]

            def w2_ap(et):
                return wp_sb[:, WP_W2 + et * 128:WP_W2 + (et + 1) * 128]

            def win_ap(c, kind, h):
                off = WP_WIN + c * 3 * G2 + kind * G2 + h * HD
                return wp_sb[:, off:off + HD]

            def adj_ap(kb):
                return adj_sb[:, kb * R:(kb + 1) * R]

            # ---------------- degree ----------------
            # Critical path: per-column degrees computed directly in [128,4]
            # layout (adjacency slice as stationary, ones as the N=1 moving
            # operand), so the gathered rsqrt reads back as plain 2D per-rank
            # slices — no partition transpose, no 32-column DMA chain.
            ps_degT = pssm.tile([128, ET], f32, tag="sm")
            for mt in range(ET):
                for kb in range(KB):
                    nc.tensor.matmul(
                        ps_degT[:, mt:mt + 1],
                        adj_sb[:, kb * R + mt * 128:kb * R + (mt + 1) * 128],
                        ones_b[:],
                        start=(kb == 0), stop=(kb == KB - 1),
                        skip_group_check=True)
            sqT = smp.tile([128, ET], f32, tag="sqT")
            nc.scalar.activation(sqT[:], ps_degT[:], AF.Sqrt)
            dcol_own = bigp.tile([128, ET], f32, tag="dcolown")
            nc.vector.reciprocal(dcol_own[:], sqT[:])

            # AG1: gather 1/sqrt(deg) across cores (column-major blocks)
            dg_in = drp.tile([128, ET], f32, tag="dgin")
            dg_out = nc.dram_tensor("dg_out", [NC_ * 128, ET], f32,
                                    kind="Internal", addr_space="Shared")
            nc.sync.dma_start(dg_in[:], dcol_own[:])
            if sim1:
                nc.sync.dma_start(dg_out[0:128, :], dg_in[:])
            else:
                nc.gpsimd.collective_compute(
                    "AllGather", ALU.bypass, replica_groups=RG,
                    ins=[dg_in.opt()], outs=[dg_out.ap()])
            dcol = bigp.tile([128, KB], f32, tag="dcol")
            for rr in range(NC_):
                nc.sync.dma_start(dcol[:, rr * ET:(rr + 1) * ET],
                                  dg_out[rr * 128:(rr + 1) * 128, :])
            # params can load behind the degree chain (SP queue order)
            nc.sync.dma_start(bp_sb[:], bp_d[:, :])
            nc.sync.dma_start(wp_sb[:], wp_d[:, :])

            # local row-degree path (off critical path): dloc feeds only the
            # own-column dbc scale used after GCN1/GCN2
            ps_deg = pssm.tile([1, R], f32, tag="sm")
            for kb in range(KB):
                nc.tensor.matmul(ps_deg[:], ones_b[:], adj_ap(kb),
                                 start=(kb == 0), stop=(kb == KB - 1))
            sq = smp.tile([1, R], f32, tag="sq")
            nc.scalar.activation(sq[:], ps_deg[:], AF.Sqrt)
            dloc = bigp.tile([1, R], f32, tag="dloc")
            nc.vector.reciprocal(dloc[:], sq[:])
            # pull the Exp ACT-table load off the first-attention critical
            # path: a dummy exp here makes bacc place the table switch now,
            # while the Activation engine is otherwise idle.
            expwarm = smp.tile([1, 1], f32, tag="expwarm")
            nc.scalar.activation(expwarm[:], sq[0:1, 0:1], AF.Exp)
            # own-column scale broadcast [128, R]
            dbc = bigp.tile([128, R], f32, tag="dbc")
            nc.gpsimd.partition_broadcast(dbc[:], dloc[:])

            # ---------------- x scale + GCN1 ----------------
            xs_sb = bigp.tile([128, KB * F_IN], bf16, tag="xs")
            for kb in range(KB):
                nc.vector.tensor_scalar_mul(
                    xs_sb[:, kb * F_IN:(kb + 1) * F_IN],
                    xp_sb[:, kb * F_IN:(kb + 1) * F_IN],
                    dcol[:, kb:kb + 1])
            ps_s1 = psqkv.tile([128, R], f32, tag="qkv")
            for kb in range(KB):
                nc.tensor.matmul(ps_s1[:], xs_sb[:, kb * F_IN:(kb + 1) * F_IN],
                                 adj_ap(kb), start=(kb == 0), stop=(kb == KB - 1))
            s1t = bigp.tile([128, R], bf16, tag="s1t")
            nc.vector.tensor_mul(s1t[:], ps_s1[:], dbc[:])
            # x1 = relu(s1.T @ W1 + b1), tile-packed [128, 4*128]
            psx = psqkv.tile([128, R], f32, tag="qkv")
            for mt in range(ET):
                nc.tensor.matmul(psx[:, mt * 128:(mt + 1) * 128],
                                 s1t[:, mt * 128:(mt + 1) * 128],
                                 w1_ap(), start=True, stop=True,
                                 skip_group_check=True)
            x1sb = bigp.tile([128, R], bf16, tag="x1sb")
            for mt in range(ET):
                tmp = smp.tile([128, G1], f32, tag="x1tmp")
                nc.vector.tensor_add(tmp[:], psx[:, mt * 128:(mt + 1) * 128],
                                     bp_sb[:, BP_B1:BP_B1 + G1])
                nc.scalar.activation(x1sb[:, mt * 128:(mt + 1) * 128], tmp[:],
                                     AF.Relu)

            # AG2: gather x1 (tile-packed per rank), pipelined in two column
            # halves (half = node-chunks {0,1} / {2,3} of every rank) so GCN2
            # can start on the first half while the second is still in flight.
            HC = R // 2
            x1g = bigp.tile([128, KB * G1], bf16, tag="x1g")
            for half in range(2):
                cs = half * HC
                x1_in = drp.tile([128, HC], bf16, tag=f"x1in{half}")
                x1_out = nc.dram_tensor(f"x1_out{half}", [NC_ * 128, HC], bf16,
                                        kind="Internal", addr_space="Shared")
                nc.scalar.dma_start(x1_in[:], x1sb[:, cs:cs + HC])
                if sim1:
                    nc.scalar.dma_start(x1_out[0:128, :], x1_in[:])
                else:
                    nc.gpsimd.collective_compute(
                        "AllGather", ALU.bypass, replica_groups=RG,
                        ins=[x1_in.opt()], outs=[x1_out.ap()])
                # per-rank 2D slice reads (proven DMA family on HW)
                for rr in range(NC_):
                    nc.sync.dma_start(
                        x1g[:, rr * R + cs:rr * R + cs + HC],
                        x1_out[rr * 128:(rr + 1) * 128, :])

            # ---------------- GCN2 ----------------
            # kb order follows AG2 half arrival: chunks {4r,4r+1} then {4r+2,4r+3}
            kb_order = [r * ET + mt for half in range(2)
                        for r in range(NC_) for mt in (half * 2, half * 2 + 1)]
            x1s = bigp.tile([128, KB * G1], bf16, tag="x1s")
            for kb in kb_order:
                nc.vector.tensor_scalar_mul(
                    x1s[:, kb * G1:(kb + 1) * G1],
                    x1g[:, kb * G1:(kb + 1) * G1],
                    dcol[:, kb:kb + 1])
            ps_s2 = psqkv.tile([128, R], f32, tag="qkv")
            for n, kb in enumerate(kb_order):
                nc.tensor.matmul(ps_s2[:], x1s[:, kb * G1:(kb + 1) * G1],
                                 adj_ap(kb), start=(n == 0), stop=(n == KB - 1))
            s2t = bigp.tile([128, R], bf16, tag="s2t")
            nc.vector.tensor_mul(s2t[:], ps_s2[:], dbc[:])
            # x2.T tiles [e-chunk 128, node 512] (b2 folded into QKV biases);
            # two 2-bank score-ring tiles so all 4 matmuls+copies pipeline.
            x2t_sb = []
            for half in range(2):
                ps2 = psbig.tile([128, 2 * R], f32, tag="sc")
                for i in range(2):
                    nc.tensor.matmul(ps2[:, i * R:(i + 1) * R],
                                     w2_ap(half * 2 + i), s2t[:],
                                     start=True, stop=True,
                                     skip_group_check=True)
                for i in range(2):
                    et = half * 2 + i
                    xt = bigp.tile([128, R], bf16, tag=f"x2_{et}")
                    if i == 0:
                        nc.vector.tensor_copy(xt[:], ps2[:, i * R:(i + 1) * R])
                    else:
                        nc.scalar.activation(xt[:], ps2[:, i * R:(i + 1) * R],
                                             AF.Identity)
                    x2t_sb.append(xt)

            # ---------------- QKV + per-head AllGather ----------------
            qt_sb = {}
            kv_out = {}
            for h in range(HEADS):
                # K^T and V share one 2-bank tile; Q reuses K's half after
                # the K bias-add has drained it.
                kvq = psqkv.tile([128, 2 * R], f32, tag="qkv")
                psk = kvq[:, 0:R]
                psv = kvq[:, R:2 * R]
                for c in range(ET):
                    nc.tensor.matmul(psk, win_ap(c, 1, h), x2t_sb[c][:],
                                     start=(c == 0), stop=(c == ET - 1),
                                     skip_group_check=True)
                kvsb = smqp.tile([128, 2 * R], bf16, tag="kvloc")
                nc.vector.tensor_scalar_add(kvsb[:, 0:R], psk,
                                            bp_sb[:, BP_BK + h:BP_BK + h + 1])
                # V tile-packed [128, 4*128] (no bias: folded into out bias)
                for mt in range(ET):
                    for c in range(ET):
                        nc.tensor.matmul(
                            psv[:, mt * HD:(mt + 1) * HD],
                            x2t_sb[c][:, mt * 128:(mt + 1) * 128],
                            win_ap(c, 2, h),
                            start=(c == 0), stop=(c == ET - 1),
                            skip_group_check=True)
                nc.vector.tensor_copy(kvsb[:, R:2 * R], psv)
                # bounce + AllGather: rank block = [KT(128) ; Vpack(128)]
                kvi = drp.tile([256, R], bf16, tag=f"kvi{h}")
                kvo = nc.dram_tensor(f"kvo{h}", [NC_ * 256, R], bf16,
                                     kind="Internal", addr_space="Shared")
                nc.scalar.dma_start(
                    kvi[:, :].rearrange("(t p) j -> p t j", t=2),
                    kvsb[:].rearrange("p (t j) -> p t j", t=2))
                if sim1:
                    nc.scalar.dma_start(kvo[0:256, :], kvi[:])
                else:
                    nc.gpsimd.collective_compute(
                        "AllGather", ALU.bypass, replica_groups=RG,
                        ins=[kvi.opt()], outs=[kvo.ap()])
                kv_out[h] = kvo
                # Q^T [d, own nodes] (overlaps the AllGather); own ring slot
                # (the ring's WAR dep serializes it behind kt/vcopy reads)
                qq = psqkv.tile([128, 2 * R], f32, tag="qkv")
                psq = qq[:, 0:R]
                for c in range(ET):
                    nc.tensor.matmul(psq, win_ap(c, 0, h), x2t_sb[c][:],
                                     start=(c == 0), stop=(c == ET - 1),
                                     skip_group_check=True)
                qt = kvp.tile([128, R], bf16, tag="qt")
                nc.vector.tensor_scalar_add(qt[:], psq,
                                            bp_sb[:, BP_BQ + h:BP_BQ + h + 1])
                qt_sb[h] = qt
                # gathered K/V loads issued here (sync queue) so earlier
                # heads' loads aren't stuck behind later heads' bounce writes
                kv3 = kvo[:, :].rearrange("(r t p) j -> t p r j", t=2, p=128)
                ktg = kvp.tile([128, NC_ * R], bf16, tag="ktg")
                nc.sync.dma_start(
                    ktg[:].rearrange("p (r j) -> p r j", r=NC_), kv3[0])
                vg = kvp.tile([128, NC_ * R], bf16, tag="vg")
                nc.sync.dma_start(
                    vg[:].rearrange("p (r j) -> p r j", r=NC_), kv3[1])
                kv_out[h] = (ktg, vg)

            # ---------------- attention per head ----------------
            # NOTE: start_tensor_calc clears has_written for the WHOLE PSUM
            # bank, so accumulation groups must not interleave within a bank
            # (a later group's start turns an earlier group's next accumulate
            # into an overwrite). u is therefore accumulated in the tail with
            # strictly sequential groups.
            zhold = []
            GRP = 2            # key chunks per score tile
            NG = KB // GRP     # 16 groups
            for h in range(HEADS):
                ktg, vg = kv_out[h]
                ps_ctx = psctx.tile([128, R], f32, tag="ctx")
                acc = smqp.tile([128, GRP * R], bf16, tag="acc")
                # software-pipelined: PV of group g is emitted alongside the
                # score matmuls of group g+2, so the in-order PE queue never
                # blocks on an exp that hasn't finished.
                SKEW = 2
                pts = {}
                for g in range(NG + SKEW):
                    if g < NG:
                        psc = psbig.tile([128, GRP * R], f32, tag="sc")
                        for i in range(GRP):
                            kc = g * GRP + i
                            nc.tensor.matmul(psc[:, i * R:(i + 1) * R],
                                             ktg[:, kc * 128:(kc + 1) * 128],
                                             qt_sb[h][:], start=True, stop=True,
                                             skip_group_check=True)
                        pt = ptp.tile([128, GRP * R], bf16, tag="pt")
                        nc.scalar.activation(pt[:], psc[:], AF.Exp,
                                             scale=inv_sqrt_hd)
                        pts[g] = pt
                        if g == 0:
                            nc.vector.tensor_copy(acc[:], pt[:])
                        else:
                            nc.vector.tensor_add(acc[:], acc[:], pt[:])
                    gp = g - SKEW
                    if gp >= 0:
                        pt = pts.pop(gp)
                        for i in range(GRP):
                            kc = gp * GRP + i
                            nc.tensor.matmul(ps_ctx[:],
                                             vg[:, kc * 128:(kc + 1) * 128],
                                             pt[:, i * R:(i + 1) * R],
                                             start=(gp == 0 and i == 0),
                                             stop=(gp == NG - 1 and i == GRP - 1),
                                             skip_group_check=True)
                # denominator: partition-reduce acc via ones matmul
                ps_den = pssm.tile([1, R], f32, tag="sm")
                for i in range(GRP):
                    nc.tensor.matmul(ps_den[:], ones_b[:],
                                     acc[:, i * R:(i + 1) * R],
                                     start=(i == 0), stop=(i == GRP - 1),
                                     skip_group_check=True)
                rden = smp.tile([1, R], f32, tag="rden")
                nc.vector.reciprocal(rden[:], ps_den[:])
                rbc = smp.tile([128, R], f32, tag="rbc")
                nc.gpsimd.partition_broadcast(rbc[:], rden[:])
                ctxn = smp.tile([128, R], f32, tag="ctxn")
                nc.vector.tensor_mul(ctxn[:], ps_ctx[:], rbc[:])
                zf = smqp.tile([128, 1], f32, tag=f"z{h}")
                nc.vector.tensor_reduce(zf[:], ctxn[:], axis=AX.X, op=ALU.add)
                zhold.append(zf)

            # ---------------- out_proj + mean + fc (partial) ----------------
            psu = psqkv.tile([128, ET], f32, tag="qkv")
            u_sb = []
            for et in range(ET):
                for h in range(HEADS):
                    wo_c = bp_sb[:, BP_WO + h * G2 + et * 128:
                                 BP_WO + h * G2 + (et + 1) * 128]
                    nc.tensor.matmul(psu[:, et:et + 1], wo_c, zhold[h][:],
                                     start=(h == 0), stop=(h == HEADS - 1),
                                     skip_group_check=True)
                ut = smp.tile([128, 1], f32, tag=f"u{et}")
                nc.vector.tensor_scalar_mul(ut[:], psu[:, et:et + 1],
                                            1.0 / float(N))
                nc.vector.tensor_add(ut[:], ut[:],
                                     bp_sb[:, BP_BO + et:BP_BO + et + 1])
                u_sb.append(ut)
            ps_fc = pssm.tile([1, 2], f32, tag="sm")
            for c in range(ET):
                nc.tensor.matmul(ps_fc[:], u_sb[c][:],
                                 bp_sb[:, BP_FCW + c * 2:BP_FCW + (c + 1) * 2],
                                 start=(c == 0), stop=(c == ET - 1))
            ores = smp.tile([1, 2], f32, tag="ores")
            nc.vector.tensor_add(ores[:], ps_fc[:],
                                 bp_sb[0:1, BP_FCB:BP_FCB + 2])
            nc.sync.dma_start(out_d[:, :], ores[:])

    nc.compile()
    return nc


def _pack_tiles(a, p=128):
    """[n*p, c] row-major -> [p, n*c] tile-packed (partition = row % p)."""
    n = a.shape[0] // p
    return np.ascontiguousarray(
        a.reshape(n, p, a.shape[1]).transpose(1, 0, 2).reshape(p, -1))


def kernel(**inputs):
    from concourse.bass_utils import run_bass_kernel_spmd

    if "nc" not in _cache:
        _cache["nc"] = _build()
    nc = _cache["nc"]

    bf = ml_dtypes.bfloat16
    adj = np.ascontiguousarray(inputs["adj_matrix"], dtype=np.float32)
    x = np.ascontiguousarray(inputs["node_features"], dtype=np.float32)
    w1 = np.asarray(inputs["W1"], np.float32)
    b1 = np.asarray(inputs["b1"], np.float32)
    w2 = np.asarray(inputs["W2"], np.float32)
    b2 = np.asarray(inputs["b2"], np.float32)
    win = np.asarray(inputs["in_proj_w"], np.float32)
    bin_ = np.asarray(inputs["in_proj_b"], np.float32)
    wo = np.asarray(inputs["out_proj_w"], np.float32)
    bo = np.asarray(inputs["out_proj_b"], np.float32)
    fcw = np.asarray(inputs["fc_w"], np.float32)
    fcb = np.asarray(inputs["fc_b"], np.float32)

    # ---- constant-parameter folding (host, exact math on fp32 weights) ----
    # x2 is only consumed by the QKV projections, so fold b2 through them.
    bq_eff = b2 @ win[:, 0:G2] + bin_[0:G2]
    bk_eff = b2 @ win[:, G2:2 * G2] + bin_[G2:2 * G2]
    bv_eff = b2 @ win[:, 2 * G2:3 * G2] + bin_[2 * G2:3 * G2]
    # V bias passes through softmax untouched (weights sum to 1), so it adds
    # bv_eff @ wo to every row of attn_out; fold into the mean+out_proj bias.
    bo_eff8 = (bo + bv_eff @ wo) / NC_

    wp = np.concatenate([
        w1, w2, _pack_tiles(win)], axis=1).astype(bf)
    bp = np.zeros((128, BP_COLS), np.float32)
    bp[:, BP_BQ:BP_BQ + 4] = bq_eff.reshape(4, 128).T
    bp[:, BP_BK:BP_BK + 4] = bk_eff.reshape(4, 128).T
    bp[:, BP_B1:BP_B1 + G1] = np.broadcast_to(b1, (128, G1))
    bp[:, BP_BO:BP_BO + 4] = bo_eff8.reshape(4, 128).T
    bp[:, BP_WO:BP_WO + ET * G2] = _pack_tiles(wo)
    bp[:, BP_FCW:BP_FCW + 8] = _pack_tiles(fcw)
    bp[0, BP_FCB:BP_FCB + 2] = fcb / NC_

    xp = _pack_tiles(x).astype(bf)
    reps = {"xp": xp, "wp": wp, "bp": bp}

    in_maps = []
    idx = np.arange(R)
    for r in range(NC_):
        cols = np.ascontiguousarray(adj[:, r * R:(r + 1) * R])
        cols[r * R + idx, idx] += 1.0   # A + I, this core's diagonal block
        in_maps.append({"adjp": _pack_tiles(cols).astype(bf), **reps})

    res = run_bass_kernel_spmd(nc, in_maps, core_ids=list(range(NC_)))
    out = np.zeros(2, dtype=np.float64)
    for r in range(NC_):
        out += res.results[r]["outp"].reshape(2).astype(np.float64)
    return out.astype(np.float32)


# revision 42
# speedup vs baseline: 1.0851x; 1.0238x over previous
"""Trainium2 Bass kernel for GCN(x2) + MHA + mean + FC, sharded over 8 NeuronCores.

Sharding: 1D row partition of the 4096 nodes (512 rows/core). Each core holds
the column slice adj_hat[:, r*512:(r+1)*512] of the symmetric A+I (by symmetry
equal to its row block transposed), all of x, and replicated weights.
Cross-core exchanges (on-device AllGather): degree vector, GCN1 output, K/V.

Key layout choices (v2):
  - All big operands are host-packed into "tile-major" [128, C] DRAM arrays so
    every load is ONE large contiguous-per-partition DMA (the HWDGE fixed
    overhead is serialized across DMAs, so DMA count dominates; the baseline's
    260 DMAs are cut to ~40).
  - Activations stay in [feature, node] / tile-packed layouts so no transposes
    are needed; matmuls run in bf16 with fp32 PSUM accumulation.
  - Constant-bias folding on host: b2 is folded into the Q/K/V biases, the V
    bias and the out_proj bias are folded together, so the device adds biases
    only where they vary per partition.
  - Softmax: exp on ACT in [128,1024] batches from 2-bank PSUM score tiles;
    denominator via a bf16 running sum on DVE (2x perf mode) + a tiny
    ones-matmul partition reduction; ctx stays in PSUM until the final
    normalize + query-sum reduce produces the per-head z.
  - PSUM rule learned on HW: start_tensor_calc clears has_written for the
    whole bank, so accumulation groups never interleave within a bank.
Host does only slicing/packing/dtype casts (and constant-parameter folding)
plus an 8-way sum of [2]-vector partials at the end.
"""
import sys
sys.path.insert(0, "/opt/trn_rl_repo")
import numpy as np
import ml_dtypes

N = 4096
NC_ = 8
R = N // NC_          # 512 rows per core
KB = N // 128         # 32 node chunks
F_IN = 128
G1 = 128
G2 = 512
HEADS = 4
HD = G2 // HEADS      # 128
ET = G2 // 128        # 4 tiles of the 512-dim embedding

# wp (bf16) column layout
WP_W1 = 0
WP_W2 = WP_W1 + F_IN          # 128
WP_WIN = WP_W2 + G2           # 640
WP_COLS = WP_WIN + ET * 3 * G2  # 640 + 6144 = 6784

# bp (fp32) column layout
BP_BQ = 0                      # [128,4] per-head q bias (b2 folded in)
BP_BK = 4                      # [128,4]
BP_B1 = 8                      # [128,128] b1 replicated across partitions
BP_BO = 136                    # [128,4] (bo + bv_eff@wo)/8 col-layout
BP_WO = 140                    # [128, 2048] out_proj 4 part-tiles
BP_FCW = BP_WO + ET * G2       # 2188: [128, 8] fc_w 4 part-tiles
BP_FCB = BP_FCW + 8            # 2196: [1,2] fc_b/8 (row 0)
BP_COLS = BP_FCB + 2           # 2198

_cache = {}


def _build(sim1=False):
    from concourse import bass, bacc, tile, mybir

    f32 = mybir.dt.float32
    bf16 = mybir.dt.bfloat16
    AF = mybir.ActivationFunctionType
    ALU = mybir.AluOpType
    AX = mybir.AxisListType

    nc = bacc.Bacc("TRN2", target_bir_lowering=False, debug=False,
                   num_devices=1 if sim1 else NC_)

    adj_d = nc.dram_tensor("adjp", [128, KB * R], bf16, kind="ExternalInput")
    x_d = nc.dram_tensor("xp", [128, KB * F_IN], bf16, kind="ExternalInput")
    wp_d = nc.dram_tensor("wp", [128, WP_COLS], bf16, kind="ExternalInput")
    bp_d = nc.dram_tensor("bp", [128, BP_COLS], f32, kind="ExternalInput")
    out_d = nc.dram_tensor("outp", [1, 2], f32, kind="ExternalOutput")

    RG = [list(range(NC_))]
    inv_sqrt_hd = 1.0 / float(np.sqrt(HD))

    with tile.TileContext(nc) as tc:
        with tc.tile_pool(name="psbig", bufs=2, space="PSUM") as psbig, \
             tc.tile_pool(name="psqkv", bufs=1, space="PSUM") as psqkv, \
             tc.tile_pool(name="psctx", bufs=1, space="PSUM") as psctx, \
             tc.tile_pool(name="pssm", bufs=1, space="PSUM") as pssm, \
             tc.tile_pool(name="big", bufs=1) as bigp, \
             tc.tile_pool(name="kv", bufs=2) as kvp, \
             tc.tile_pool(name="pt", bufs=3) as ptp, \
             tc.tile_pool(name="sm", bufs=2) as smp, \
             tc.tile_pool(name="smq", bufs=2) as smqp, \
             tc.tile_pool(name="dram", bufs=1, space="DRAM") as drp:

            # ---------------- constants / params ----------------
            ones_b = bigp.tile([128, 1], bf16, tag="ones_b")
            nc.vector.memset(ones_b[:], 1.0)

            # All big loads go on the SP (sync) queue in priority order —
            # SEQs issue in order, so this fixes the DMA_ENGINES schedule:
            # adjacency (gates degree) > x > [degree-chain DMAs] > biases >
            # weights (first needed by the W1 matmul, well after GCN1).
            adj_sb = bigp.tile([128, KB * R], bf16, tag="adj")
            NLOAD = 8
            CW = KB * R // NLOAD
            for k in range(NLOAD):
                nc.sync.dma_start(adj_sb[:, k * CW:(k + 1) * CW],
                                  adj_d[:, k * CW:(k + 1) * CW])
            xp_sb = bigp.tile([128, KB * F_IN], bf16, tag="xp")
            nc.sync.dma_start(xp_sb[:], x_d[:, :])
            wp_sb = bigp.tile([128, WP_COLS], bf16, tag="wp")
            bp_sb = bigp.tile([128, BP_COLS], f32, tag="bp")

            def w1_ap():
                return wp_sb[:, WP_W1:WP_W1 + G1]

            def w2_ap(et):
                return wp_sb[:, WP_W2 + et * 128:WP_W2 + (et + 1) * 128]

            def win_ap(c, kind, h):
                off = WP_WIN + c * 3 * G2 + kind * G2 + h * HD
                return wp_sb[:, off:off + HD]

            def adj_ap(kb):
                return adj_sb[:, kb * R:(kb + 1) * R]

            # ---------------- degree ----------------
            # Critical path: per-column degrees computed directly in [128,4]
            # layout (adjacency slice as stationary, ones as the N=1 moving
            # operand), so the gathered rsqrt reads back as plain 2D per-rank
            # slices — no partition transpose, no 32-column DMA chain.
            ps_degT = pssm.tile([128, ET], f32, tag="sm")
            for mt in range(ET):
                for kb in range(KB):
                    nc.tensor.matmul(
                        ps_degT[:, mt:mt + 1],
                        adj_sb[:, kb * R + mt * 128:kb * R + (mt + 1) * 128],
                        ones_b[:],
                        start=(kb == 0), stop=(kb == KB - 1),
                        skip_group_check=True)
            sqT = smp.tile([128, ET], f32, tag="sqT")
            nc.scalar.activation(sqT[:], ps_degT[:], AF.Sqrt)
            dcol_own = bigp.tile([128, ET], f32, tag="dcolown")
            nc.vector.reciprocal(dcol_own[:], sqT[:])

            # AG1: gather 1/sqrt(deg) across cores (column-major blocks)
            dg_in = drp.tile([128, ET], f32, tag="dgin")
            dg_out = nc.dram_tensor("dg_out", [NC_ * 128, ET], f32,
                                    kind="Internal", addr_space="Shared")
            nc.sync.dma_start(dg_in[:], dcol_own[:])
            if sim1:
                nc.sync.dma_start(dg_out[0:128, :], dg_in[:])
            else:
                nc.gpsimd.collective_compute(
                    "AllGather", ALU.bypass, replica_groups=RG,
                    ins=[dg_in.opt()], outs=[dg_out.ap()])
            dcol = bigp.tile([128, KB], f32, tag="dcol")
            for rr in range(NC_):
                nc.sync.dma_start(dcol[:, rr * ET:(rr + 1) * ET],
                                  dg_out[rr * 128:(rr + 1) * 128, :])
            # params can load behind the degree chain (SP queue order)
            nc.sync.dma_start(bp_sb[:], bp_d[:, :])
            nc.sync.dma_start(wp_sb[:], wp_d[:, :])

            # local row-degree path (off critical path): dloc feeds only the
            # own-column dbc scale used after GCN1/GCN2
            ps_deg = pssm.tile([1, R], f32, tag="sm")
            for kb in range(KB):
                nc.tensor.matmul(ps_deg[:], ones_b[:], adj_ap(kb),
                                 start=(kb == 0), stop=(kb == KB - 1))
            sq = smp.tile([1, R], f32, tag="sq")
            nc.scalar.activation(sq[:], ps_deg[:], AF.Sqrt)
            dloc = bigp.tile([1, R], f32, tag="dloc")
            nc.vector.reciprocal(dloc[:], sq[:])
            # pull the Exp ACT-table load off the first-attention critical
            # path: a dummy exp here makes bacc place the table switch now,
            # while the Activation engine is otherwise idle.
            expwarm = smp.tile([1, 1], f32, tag="expwarm")
            nc.scalar.activation(expwarm[:], sq[0:1, 0:1], AF.Exp)
            # own-column scale broadcast [128, R]
            dbc = bigp.tile([128, R], f32, tag="dbc")
            nc.gpsimd.partition_broadcast(dbc[:], dloc[:])

            # ---------------- x scale + GCN1 ----------------
            xs_sb = bigp.tile([128, KB * F_IN], bf16, tag="xs")
            for kb in range(KB):
                nc.vector.tensor_scalar_mul(
                    xs_sb[:, kb * F_IN:(kb + 1) * F_IN],
                    xp_sb[:, kb * F_IN:(kb + 1) * F_IN],
                    dcol[:, kb:kb + 1])
            ps_s1 = psqkv.tile([128, R], f32, tag="qkv")
            for kb in range(KB):
                nc.tensor.matmul(ps_s1[:], xs_sb[:, kb * F_IN:(kb + 1) * F_IN],
                                 adj_ap(kb), start=(kb == 0), stop=(kb == KB - 1))
            s1t = bigp.tile([128, R], bf16, tag="s1t")
            nc.vector.tensor_mul(s1t[:], ps_s1[:], dbc[:])
            # x1 = relu(s1.T @ W1 + b1), tile-packed [128, 4*128]
            psx = psqkv.tile([128, R], f32, tag="qkv")
            for mt in range(ET):
                nc.tensor.matmul(psx[:, mt * 128:(mt + 1) * 128],
                                 s1t[:, mt * 128:(mt + 1) * 128],
                                 w1_ap(), start=True, stop=True,
                                 skip_group_check=True)
            x1sb = bigp.tile([128, R], bf16, tag="x1sb")
            for mt in range(ET):
                tmp = smp.tile([128, G1], f32, tag="x1tmp")
                nc.vector.tensor_add(tmp[:], psx[:, mt * 128:(mt + 1) * 128],
                                     bp_sb[:, BP_B1:BP_B1 + G1])
                nc.scalar.activation(x1sb[:, mt * 128:(mt + 1) * 128], tmp[:],
                                     AF.Relu)

            # AG2: gather x1 (tile-packed per rank), pipelined in two column
            # halves (half = node-chunks {0,1} / {2,3} of every rank) so GCN2
            # can start on the first half while the second is still in flight.
            HC = R // 2
            x1g = bigp.tile([128, KB * G1], bf16, tag="x1g")
            for half in range(2):
                cs = half * HC
                x1_in = drp.tile([128, HC], bf16, tag=f"x1in{half}")
                x1_out = nc.dram_tensor(f"x1_out{half}", [NC_ * 128, HC], bf16,
                                        kind="Internal", addr_space="Shared")
                nc.scalar.dma_start(x1_in[:], x1sb[:, cs:cs + HC])
                if sim1:
                    nc.scalar.dma_start(x1_out[0:128, :], x1_in[:])
                else:
                    nc.gpsimd.collective_compute(
                        "AllGather", ALU.bypass, replica_groups=RG,
                        ins=[x1_in.opt()], outs=[x1_out.ap()])
                # per-rank 2D slice reads (proven DMA family on HW)
                for rr in range(NC_):
                    nc.sync.dma_start(
                        x1g[:, rr * R + cs:rr * R + cs + HC],
                        x1_out[rr * 128:(rr + 1) * 128, :])

            # ---------------- GCN2 ----------------
            # kb order follows AG2 arrival: halves {4r,4r+1} then {4r+2,4r+3},
            # and within each half the own-rank (0) block last — its reload is
            # gated by the full write->gather->read chain while the other
            # blocks are already in flight, so the accumulation (order-
            # independent) starts immediately on what has landed.
            kb_order = [r * ET + mt for half in range(2)
                        for r in list(range(1, NC_)) + [0]
                        for mt in (half * 2, half * 2 + 1)]
            x1s = bigp.tile([128, KB * G1], bf16, tag="x1s")
            for kb in kb_order:
                nc.vector.tensor_scalar_mul(
                    x1s[:, kb * G1:(kb + 1) * G1],
                    x1g[:, kb * G1:(kb + 1) * G1],
                    dcol[:, kb:kb + 1])
            ps_s2 = psqkv.tile([128, R], f32, tag="qkv")
            for n, kb in enumerate(kb_order):
                nc.tensor.matmul(ps_s2[:], x1s[:, kb * G1:(kb + 1) * G1],
                                 adj_ap(kb), start=(n == 0), stop=(n == KB - 1))
            s2t = bigp.tile([128, R], bf16, tag="s2t")
            nc.vector.tensor_mul(s2t[:], ps_s2[:], dbc[:])
            # x2.T tiles [e-chunk 128, node 512] (b2 folded into QKV biases);
            # two 2-bank score-ring tiles so all 4 matmuls+copies pipeline.
            x2t_sb = []
            for half in range(2):
                ps2 = psbig.tile([128, 2 * R], f32, tag="sc")
                for i in range(2):
                    nc.tensor.matmul(ps2[:, i * R:(i + 1) * R],
                                     w2_ap(half * 2 + i), s2t[:],
                                     start=True, stop=True,
                                     skip_group_check=True)
                for i in range(2):
                    et = half * 2 + i
                    xt = bigp.tile([128, R], bf16, tag=f"x2_{et}")
                    if i == 0:
                        nc.vector.tensor_copy(xt[:], ps2[:, i * R:(i + 1) * R])
                    else:
                        nc.scalar.activation(xt[:], ps2[:, i * R:(i + 1) * R],
                                             AF.Identity)
                    x2t_sb.append(xt)

            # ---------------- QKV + per-head AllGather ----------------
            qt_sb = {}
            kv_out = {}
            for h in range(HEADS):
                # K^T and V share one 2-bank tile; Q reuses K's half after
                # the K bias-add has drained it.
                kvq = psqkv.tile([128, 2 * R], f32, tag="qkv")
                psk = kvq[:, 0:R]
                psv = kvq[:, R:2 * R]
                for c in range(ET):
                    nc.tensor.matmul(psk, win_ap(c, 1, h), x2t_sb[c][:],
                                     start=(c == 0), stop=(c == ET - 1),
                                     skip_group_check=True)
                kvsb = smqp.tile([128, 2 * R], bf16, tag="kvloc")
                nc.vector.tensor_scalar_add(kvsb[:, 0:R], psk,
                                            bp_sb[:, BP_BK + h:BP_BK + h + 1])
                # V tile-packed [128, 4*128] (no bias: folded into out bias)
                for mt in range(ET):
                    for c in range(ET):
                        nc.tensor.matmul(
                            psv[:, mt * HD:(mt + 1) * HD],
                            x2t_sb[c][:, mt * 128:(mt + 1) * 128],
                            win_ap(c, 2, h),
                            start=(c == 0), stop=(c == ET - 1),
                            skip_group_check=True)
                nc.vector.tensor_copy(kvsb[:, R:2 * R], psv)
                # bounce + AllGather: rank block = [KT(128) ; Vpack(128)]
                kvi = drp.tile([256, R], bf16, tag=f"kvi{h}")
                kvo = nc.dram_tensor(f"kvo{h}", [NC_ * 256, R], bf16,
                                     kind="Internal", addr_space="Shared")
                nc.scalar.dma_start(
                    kvi[:, :].rearrange("(t p) j -> p t j", t=2),
                    kvsb[:].rearrange("p (t j) -> p t j", t=2))
                if sim1:
                    nc.scalar.dma_start(kvo[0:256, :], kvi[:])
                else:
                    nc.gpsimd.collective_compute(
                        "AllGather", ALU.bypass, replica_groups=RG,
                        ins=[kvi.opt()], outs=[kvo.ap()])
                kv_out[h] = kvo
                # Q^T [d, own nodes] (overlaps the AllGather); own ring slot
                # (the ring's WAR dep serializes it behind kt/vcopy reads)
                qq = psqkv.tile([128, 2 * R], f32, tag="qkv")
                psq = qq[:, 0:R]
                for c in range(ET):
                    nc.tensor.matmul(psq, win_ap(c, 0, h), x2t_sb[c][:],
                                     start=(c == 0), stop=(c == ET - 1),
                                     skip_group_check=True)
                qt = kvp.tile([128, R], bf16, tag="qt")
                nc.vector.tensor_scalar_add(qt[:], psq,
                                            bp_sb[:, BP_BQ + h:BP_BQ + h + 1])
                qt_sb[h] = qt
                # gathered K/V loads issued here (sync queue) so earlier
                # heads' loads aren't stuck behind later heads' bounce writes
                kv3 = kvo[:, :].rearrange("(r t p) j -> t p r j", t=2, p=128)
                ktg = kvp.tile([128, NC_ * R], bf16, tag="ktg")
                nc.sync.dma_start(
                    ktg[:].rearrange("p (r j) -> p r j", r=NC_), kv3[0])
                vg = kvp.tile([128, NC_ * R], bf16, tag="vg")
                nc.sync.dma_start(
                    vg[:].rearrange("p (r j) -> p r j", r=NC_), kv3[1])
                kv_out[h] = (ktg, vg)

            # ---------------- attention per head ----------------
            # NOTE: start_tensor_calc clears has_written for the WHOLE PSUM
            # bank, so accumulation groups must not interleave within a bank
            # (a later group's start turns an earlier group's next accumulate
            # into an overwrite). u is therefore accumulated in the tail with
            # strictly sequential groups.
            zhold = []
            GRP = 2            # key chunks per score tile
            NG = KB // GRP     # 16 groups
            for h in range(HEADS):
                ktg, vg = kv_out[h]
                ps_ctx = psctx.tile([128, R], f32, tag="ctx")
                acc = smqp.tile([128, GRP * R], bf16, tag="acc")
                # software-pipelined: PV of group g is emitted alongside the
                # score matmuls of group g+2, so the in-order PE queue never
                # blocks on an exp that hasn't finished.
                SKEW = 2
                pts = {}
                for g in range(NG + SKEW):
                    if g < NG:
                        psc = psbig.tile([128, GRP * R], f32, tag="sc")
                        for i in range(GRP):
                            kc = g * GRP + i
                            nc.tensor.matmul(psc[:, i * R:(i + 1) * R],
                                             ktg[:, kc * 128:(kc + 1) * 128],
                                             qt_sb[h][:], start=True, stop=True,
                                             skip_group_check=True)
                        pt = ptp.tile([128, GRP * R], bf16, tag="pt")
                        nc.scalar.activation(pt[:], psc[:], AF.Exp,
                                             scale=inv_sqrt_hd)
                        pts[g] = pt
                        if g == 0:
                            nc.vector.tensor_copy(acc[:], pt[:])
                        else:
                            nc.vector.tensor_add(acc[:], acc[:], pt[:])
                    gp = g - SKEW
                    if gp >= 0:
                        pt = pts.pop(gp)
                        for i in range(GRP):
                            kc = gp * GRP + i
                            nc.tensor.matmul(ps_ctx[:],
                                             vg[:, kc * 128:(kc + 1) * 128],
                                             pt[:, i * R:(i + 1) * R],
                                             start=(gp == 0 and i == 0),
                                             stop=(gp == NG - 1 and i == GRP - 1),
                                             skip_group_check=True)
                # denominator: partition-reduce acc via ones matmul
                ps_den = pssm.tile([1, R], f32, tag="sm")
                for i in range(GRP):
                    nc.tensor.matmul(ps_den[:], ones_b[:],
                                     acc[:, i * R:(i + 1) * R],
                                     start=(i == 0), stop=(i == GRP - 1),
                                     skip_group_check=True)
                rden = smp.tile([1, R], f32, tag="rden")
                nc.vector.reciprocal(rden[:], ps_den[:])
                rbc = smp.tile([128, R], f32, tag="rbc")
                nc.gpsimd.partition_broadcast(rbc[:], rden[:])
                ctxn = smp.tile([128, R], f32, tag="ctxn")
                nc.vector.tensor_mul(ctxn[:], ps_ctx[:], rbc[:])
                zf = smqp.tile([128, 1], f32, tag=f"z{h}")
                nc.vector.tensor_reduce(zf[:], ctxn[:], axis=AX.X, op=ALU.add)
                zhold.append(zf)

            # ---------------- out_proj + mean + fc (partial) ----------------
            psu = psqkv.tile([128, ET], f32, tag="qkv")
            u_sb = []
            for et in range(ET):
                for h in range(HEADS):
                    wo_c = bp_sb[:, BP_WO + h * G2 + et * 128:
                                 BP_WO + h * G2 + (et + 1) * 128]
                    nc.tensor.matmul(psu[:, et:et + 1], wo_c, zhold[h][:],
                                     start=(h == 0), stop=(h == HEADS - 1),
                                     skip_group_check=True)
                ut = smp.tile([128, 1], f32, tag=f"u{et}")
                nc.vector.tensor_scalar_mul(ut[:], psu[:, et:et + 1],
                                            1.0 / float(N))
                nc.vector.tensor_add(ut[:], ut[:],
                                     bp_sb[:, BP_BO + et:BP_BO + et + 1])
                u_sb.append(ut)
            ps_fc = pssm.tile([1, 2], f32, tag="sm")
            for c in range(ET):
                nc.tensor.matmul(ps_fc[:], u_sb[c][:],
                                 bp_sb[:, BP_FCW + c * 2:BP_FCW + (c + 1) * 2],
                                 start=(c == 0), stop=(c == ET - 1))
            ores = smp.tile([1, 2], f32, tag="ores")
            nc.vector.tensor_add(ores[:], ps_fc[:],
                                 bp_sb[0:1, BP_FCB:BP_FCB + 2])
            nc.sync.dma_start(out_d[:, :], ores[:])

    nc.compile()
    return nc


def _pack_tiles(a, p=128):
    """[n*p, c] row-major -> [p, n*c] tile-packed (partition = row % p)."""
    n = a.shape[0] // p
    return np.ascontiguousarray(
        a.reshape(n, p, a.shape[1]).transpose(1, 0, 2).reshape(p, -1))


def kernel(**inputs):
    from concourse.bass_utils import run_bass_kernel_spmd

    if "nc" not in _cache:
        _cache["nc"] = _build()
    nc = _cache["nc"]

    bf = ml_dtypes.bfloat16
    adj = np.ascontiguousarray(inputs["adj_matrix"], dtype=np.float32)
    x = np.ascontiguousarray(inputs["node_features"], dtype=np.float32)
    w1 = np.asarray(inputs["W1"], np.float32)
    b1 = np.asarray(inputs["b1"], np.float32)
    w2 = np.asarray(inputs["W2"], np.float32)
    b2 = np.asarray(inputs["b2"], np.float32)
    win = np.asarray(inputs["in_proj_w"], np.float32)
    bin_ = np.asarray(inputs["in_proj_b"], np.float32)
    wo = np.asarray(inputs["out_proj_w"], np.float32)
    bo = np.asarray(inputs["out_proj_b"], np.float32)
    fcw = np.asarray(inputs["fc_w"], np.float32)
    fcb = np.asarray(inputs["fc_b"], np.float32)

    # ---- constant-parameter folding (host, exact math on fp32 weights) ----
    # x2 is only consumed by the QKV projections, so fold b2 through them.
    bq_eff = b2 @ win[:, 0:G2] + bin_[0:G2]
    bk_eff = b2 @ win[:, G2:2 * G2] + bin_[G2:2 * G2]
    bv_eff = b2 @ win[:, 2 * G2:3 * G2] + bin_[2 * G2:3 * G2]
    # V bias passes through softmax untouched (weights sum to 1), so it adds
    # bv_eff @ wo to every row of attn_out; fold into the mean+out_proj bias.
    bo_eff8 = (bo + bv_eff @ wo) / NC_

    wp = np.concatenate([
        w1, w2, _pack_tiles(win)], axis=1).astype(bf)
    bp = np.zeros((128, BP_COLS), np.float32)
    bp[:, BP_BQ:BP_BQ + 4] = bq_eff.reshape(4, 128).T
    bp[:, BP_BK:BP_BK + 4] = bk_eff.reshape(4, 128).T
    bp[:, BP_B1:BP_B1 + G1] = np.broadcast_to(b1, (128, G1))
    bp[:, BP_BO:BP_BO + 4] = bo_eff8.reshape(4, 128).T
    bp[:, BP_WO:BP_WO + ET * G2] = _pack_tiles(wo)
    bp[:, BP_FCW:BP_FCW + 8] = _pack_tiles(fcw)
    bp[0, BP_FCB:BP_FCB + 2] = fcb / NC_

    xp = _pack_tiles(x).astype(bf)
    reps = {"xp": xp, "wp": wp, "bp": bp}

    in_maps = []
    idx = np.arange(R)
    for r in range(NC_):
        cols = np.ascontiguousarray(adj[:, r * R:(r + 1) * R])
        cols[r * R + idx, idx] += 1.0   # A + I, this core's diagonal block
        in_maps.append({"adjp": _pack_tiles(cols).astype(bf), **reps})

    res = run_bass_kernel_spmd(nc, in_maps, core_ids=list(range(NC_)))
    out = np.zeros(2, dtype=np.float64)
    for r in range(NC_):
        out += res.results[r]["outp"].reshape(2).astype(np.float64)
    return out.astype(np.float32)


# revision 48
# speedup vs baseline: 1.0963x; 1.0103x over previous
"""Trainium2 Bass kernel for GCN(x2) + MHA + mean + FC, sharded over 8 NeuronCores.

Sharding: 1D row partition of the 4096 nodes (512 rows/core). Each core holds
the column slice adj_hat[:, r*512:(r+1)*512] of the symmetric A+I (by symmetry
equal to its row block transposed), all of x, and replicated weights.
Cross-core exchanges (on-device AllGather): degree vector, GCN1 output, K/V.

Key layout choices (v2):
  - All big operands are host-packed into "tile-major" [128, C] DRAM arrays so
    every load is ONE large contiguous-per-partition DMA (the HWDGE fixed
    overhead is serialized across DMAs, so DMA count dominates; the baseline's
    260 DMAs are cut to ~40).
  - Activations stay in [feature, node] / tile-packed layouts so no transposes
    are needed; matmuls run in bf16 with fp32 PSUM accumulation.
  - Constant-bias folding on host: b2 is folded into the Q/K/V biases, the V
    bias and the out_proj bias are folded together, so the device adds biases
    only where they vary per partition.
  - Softmax: exp on ACT in [128,1024] batches from 2-bank PSUM score tiles;
    denominator via a bf16 running sum on DVE (2x perf mode) + a tiny
    ones-matmul partition reduction; ctx stays in PSUM until the final
    normalize + query-sum reduce produces the per-head z.
  - PSUM rule learned on HW: start_tensor_calc clears has_written for the
    whole bank, so accumulation groups never interleave within a bank.
Host does only slicing/packing/dtype casts (and constant-parameter folding)
plus an 8-way sum of [2]-vector partials at the end.
"""
import sys
sys.path.insert(0, "/opt/trn_rl_repo")
import numpy as np
import ml_dtypes

N = 4096
NC_ = 8
R = N // NC_          # 512 rows per core
KB = N // 128         # 32 node chunks
F_IN = 128
G1 = 128
G2 = 512
HEADS = 4
HD = G2 // HEADS      # 128
ET = G2 // 128        # 4 tiles of the 512-dim embedding

# wp (bf16) column layout
WP_W1 = 0
WP_W2 = WP_W1 + F_IN          # 128
WP_WIN = WP_W2 + G2           # 640
WP_COLS = WP_WIN + ET * 3 * G2  # 640 + 6144 = 6784

# bp (fp32) column layout
BP_BQ = 0                      # [128,4] per-head q bias (b2 folded in)
BP_BK = 4                      # [128,4]
BP_B1 = 8                      # [128,128] b1 replicated across partitions
BP_BO = 136                    # [128,4] (bo + bv_eff@wo)/8 col-layout
BP_WO = 140                    # [128, 2048] out_proj 4 part-tiles
BP_FCW = BP_WO + ET * G2       # 2188: [128, 8] fc_w 4 part-tiles
BP_FCB = BP_FCW + 8            # 2196: [1,2] fc_b/8 (row 0)
BP_COLS = BP_FCB + 2           # 2198

_cache = {}


def _build(sim1=False):
    from concourse import bass, bacc, tile, mybir

    f32 = mybir.dt.float32
    bf16 = mybir.dt.bfloat16
    AF = mybir.ActivationFunctionType
    ALU = mybir.AluOpType
    AX = mybir.AxisListType

    nc = bacc.Bacc("TRN2", target_bir_lowering=False, debug=False,
                   num_devices=1 if sim1 else NC_)

    adj_d = nc.dram_tensor("adjp", [128, KB * R], bf16, kind="ExternalInput")
    x_d = nc.dram_tensor("xp", [128, KB * F_IN], bf16, kind="ExternalInput")
    wp_d = nc.dram_tensor("wp", [128, WP_COLS], bf16, kind="ExternalInput")
    bp_d = nc.dram_tensor("bp", [128, BP_COLS], f32, kind="ExternalInput")
    out_d = nc.dram_tensor("outp", [1, 2], f32, kind="ExternalOutput")

    RG = [list(range(NC_))]
    inv_sqrt_hd = 1.0 / float(np.sqrt(HD))

    with tile.TileContext(nc) as tc:
        with tc.tile_pool(name="psbig", bufs=2, space="PSUM") as psbig, \
             tc.tile_pool(name="psqkv", bufs=1, space="PSUM") as psqkv, \
             tc.tile_pool(name="psctx", bufs=1, space="PSUM") as psctx, \
             tc.tile_pool(name="pssm", bufs=1, space="PSUM") as pssm, \
             tc.tile_pool(name="big", bufs=1) as bigp, \
             tc.tile_pool(name="kv", bufs=2) as kvp, \
             tc.tile_pool(name="pt", bufs=4) as ptp, \
             tc.tile_pool(name="sm", bufs=2) as smp, \
             tc.tile_pool(name="smq", bufs=2) as smqp, \
             tc.tile_pool(name="dram", bufs=1, space="DRAM") as drp:

            # ---------------- constants / params ----------------
            ones_b = bigp.tile([128, 1], bf16, tag="ones_b")
            nc.vector.memset(ones_b[:], 1.0)

            # All big loads go on the SP (sync) queue in priority order —
            # SEQs issue in order, so this fixes the DMA_ENGINES schedule:
            # adjacency (gates degree) > x > [degree-chain DMAs] > biases >
            # weights (first needed by the W1 matmul, well after GCN1).
            adj_sb = bigp.tile([128, KB * R], bf16, tag="adj")
            NLOAD = 8
            CW = KB * R // NLOAD
            for k in range(NLOAD):
                nc.sync.dma_start(adj_sb[:, k * CW:(k + 1) * CW],
                                  adj_d[:, k * CW:(k + 1) * CW])
            xp_sb = bigp.tile([128, KB * F_IN], bf16, tag="xp")
            nc.sync.dma_start(xp_sb[:], x_d[:, :])
            wp_sb = bigp.tile([128, WP_COLS], bf16, tag="wp")
            bp_sb = bigp.tile([128, BP_COLS], f32, tag="bp")

            def w1_ap():
                return wp_sb[:, WP_W1:WP_W1 + G1]

            def w2_ap(et):
                return wp_sb[:, WP_W2 + et * 128:WP_W2 + (et + 1) * 128]

            def win_ap(c, kind, h):
                off = WP_WIN + c * 3 * G2 + kind * G2 + h * HD
                return wp_sb[:, off:off + HD]

            def adj_ap(kb):
                return adj_sb[:, kb * R:(kb + 1) * R]

            # ---------------- degree ----------------
            # Critical path: per-column degrees computed directly in [128,4]
            # layout (adjacency slice as stationary, ones as the N=1 moving
            # operand), so the gathered rsqrt reads back as plain 2D per-rank
            # slices — no partition transpose, no 32-column DMA chain.
            ps_degT = pssm.tile([128, ET], f32, tag="sm")
            for mt in range(ET):
                for kb in range(KB):
                    nc.tensor.matmul(
                        ps_degT[:, mt:mt + 1],
                        adj_sb[:, kb * R + mt * 128:kb * R + (mt + 1) * 128],
                        ones_b[:],
                        start=(kb == 0), stop=(kb == KB - 1),
                        skip_group_check=True)
            sqT = smp.tile([128, ET], f32, tag="sqT")
            nc.scalar.activation(sqT[:], ps_degT[:], AF.Sqrt)
            dcol_own = bigp.tile([128, ET], f32, tag="dcolown")
            nc.vector.reciprocal(dcol_own[:], sqT[:])

            # AG1: gather 1/sqrt(deg) across cores (column-major blocks)
            dg_in = drp.tile([128, ET], f32, tag="dgin")
            dg_out = nc.dram_tensor("dg_out", [NC_ * 128, ET], f32,
                                    kind="Internal", addr_space="Shared")
            nc.sync.dma_start(dg_in[:], dcol_own[:])
            if sim1:
                nc.sync.dma_start(dg_out[0:128, :], dg_in[:])
            else:
                nc.gpsimd.collective_compute(
                    "AllGather", ALU.bypass, replica_groups=RG,
                    ins=[dg_in.opt()], outs=[dg_out.ap()])
            dcol = bigp.tile([128, KB], f32, tag="dcol")
            for rr in range(NC_):
                nc.sync.dma_start(dcol[:, rr * ET:(rr + 1) * ET],
                                  dg_out[rr * 128:(rr + 1) * 128, :])
            # params can load behind the degree chain (SP queue order)
            nc.sync.dma_start(bp_sb[:], bp_d[:, :])
            nc.sync.dma_start(wp_sb[:], wp_d[:, :])

            # local row-degree path (off critical path): dloc feeds only the
            # own-column dbc scale used after GCN1/GCN2
            ps_deg = pssm.tile([1, R], f32, tag="sm")
            for kb in range(KB):
                nc.tensor.matmul(ps_deg[:], ones_b[:], adj_ap(kb),
                                 start=(kb == 0), stop=(kb == KB - 1))
            sq = smp.tile([1, R], f32, tag="sq")
            nc.scalar.activation(sq[:], ps_deg[:], AF.Sqrt)
            dloc = bigp.tile([1, R], f32, tag="dloc")
            nc.vector.reciprocal(dloc[:], sq[:])
            # pull the Exp ACT-table load off the first-attention critical
            # path: a dummy exp here makes bacc place the table switch now,
            # while the Activation engine is otherwise idle.
            expwarm = smp.tile([1, 1], f32, tag="expwarm")
            nc.scalar.activation(expwarm[:], sq[0:1, 0:1], AF.Exp)
            # own-column scale broadcast [128, R]
            dbc = bigp.tile([128, R], f32, tag="dbc")
            nc.gpsimd.partition_broadcast(dbc[:], dloc[:])

            # ---------------- x scale + GCN1 ----------------
            xs_sb = bigp.tile([128, KB * F_IN], bf16, tag="xs")
            for kb in range(KB):
                nc.vector.tensor_scalar_mul(
                    xs_sb[:, kb * F_IN:(kb + 1) * F_IN],
                    xp_sb[:, kb * F_IN:(kb + 1) * F_IN],
                    dcol[:, kb:kb + 1])
            ps_s1 = psqkv.tile([128, R], f32, tag="qkv")
            for kb in range(KB):
                nc.tensor.matmul(ps_s1[:], xs_sb[:, kb * F_IN:(kb + 1) * F_IN],
                                 adj_ap(kb), start=(kb == 0), stop=(kb == KB - 1))
            s1t = bigp.tile([128, R], bf16, tag="s1t")
            nc.vector.tensor_mul(s1t[:], ps_s1[:], dbc[:])
            # x1 = relu(s1.T @ W1 + b1), tile-packed [128, 4*128]
            psx = psqkv.tile([128, R], f32, tag="qkv")
            for mt in range(ET):
                nc.tensor.matmul(psx[:, mt * 128:(mt + 1) * 128],
                                 s1t[:, mt * 128:(mt + 1) * 128],
                                 w1_ap(), start=True, stop=True,
                                 skip_group_check=True)
            x1sb = bigp.tile([128, R], bf16, tag="x1sb")
            for mt in range(ET):
                tmp = smp.tile([128, G1], f32, tag="x1tmp")
                nc.vector.tensor_add(tmp[:], psx[:, mt * 128:(mt + 1) * 128],
                                     bp_sb[:, BP_B1:BP_B1 + G1])
                nc.scalar.activation(x1sb[:, mt * 128:(mt + 1) * 128], tmp[:],
                                     AF.Relu)

            # AG2: gather x1 (tile-packed per rank), pipelined in two column
            # halves (half = node-chunks {0,1} / {2,3} of every rank) so GCN2
            # can start on the first half while the second is still in flight.
            HC = R // 2
            x1g = bigp.tile([128, KB * G1], bf16, tag="x1g")
            for half in range(2):
                cs = half * HC
                x1_in = drp.tile([128, HC], bf16, tag=f"x1in{half}")
                x1_out = nc.dram_tensor(f"x1_out{half}", [NC_ * 128, HC], bf16,
                                        kind="Internal", addr_space="Shared")
                nc.scalar.dma_start(x1_in[:], x1sb[:, cs:cs + HC])
                if sim1:
                    nc.scalar.dma_start(x1_out[0:128, :], x1_in[:])
                else:
                    nc.gpsimd.collective_compute(
                        "AllGather", ALU.bypass, replica_groups=RG,
                        ins=[x1_in.opt()], outs=[x1_out.ap()])
                # per-rank 2D slice reads (proven DMA family on HW)
                for rr in range(NC_):
                    nc.sync.dma_start(
                        x1g[:, rr * R + cs:rr * R + cs + HC],
                        x1_out[rr * 128:(rr + 1) * 128, :])

            # ---------------- GCN2 ----------------
            # kb order follows AG2 arrival: halves {4r,4r+1} then {4r+2,4r+3},
            # and within each half the own-rank (0) block last — its reload is
            # gated by the full write->gather->read chain while the other
            # blocks are already in flight, so the accumulation (order-
            # independent) starts immediately on what has landed.
            kb_order = [r * ET + mt for half in range(2)
                        for r in list(range(1, NC_)) + [0]
                        for mt in (half * 2, half * 2 + 1)]
            x1s = bigp.tile([128, KB * G1], bf16, tag="x1s")
            for kb in kb_order:
                nc.vector.tensor_scalar_mul(
                    x1s[:, kb * G1:(kb + 1) * G1],
                    x1g[:, kb * G1:(kb + 1) * G1],
                    dcol[:, kb:kb + 1])
            ps_s2 = psqkv.tile([128, R], f32, tag="qkv")
            for n, kb in enumerate(kb_order):
                nc.tensor.matmul(ps_s2[:], x1s[:, kb * G1:(kb + 1) * G1],
                                 adj_ap(kb), start=(n == 0), stop=(n == KB - 1))
            s2t = bigp.tile([128, R], bf16, tag="s2t")
            nc.vector.tensor_mul(s2t[:], ps_s2[:], dbc[:])
            # x2.T tiles [e-chunk 128, node 512] (b2 folded into QKV biases);
            # two 2-bank score-ring tiles so all 4 matmuls+copies pipeline.
            x2t_sb = []
            for half in range(2):
                ps2 = psbig.tile([128, 2 * R], f32, tag="sc")
                for i in range(2):
                    nc.tensor.matmul(ps2[:, i * R:(i + 1) * R],
                                     w2_ap(half * 2 + i), s2t[:],
                                     start=True, stop=True,
                                     skip_group_check=True)
                for i in range(2):
                    et = half * 2 + i
                    xt = bigp.tile([128, R], bf16, tag=f"x2_{et}")
                    if i == 0:
                        nc.vector.tensor_copy(xt[:], ps2[:, i * R:(i + 1) * R])
                    else:
                        nc.scalar.activation(xt[:], ps2[:, i * R:(i + 1) * R],
                                             AF.Identity)
                    x2t_sb.append(xt)

            # ---------------- QKV + per-head AllGather ----------------
            qt_sb = {}
            kv_out = {}
            for h in range(HEADS):
                # K^T and V share one 2-bank tile; Q reuses K's half after
                # the K bias-add has drained it.
                kvq = psqkv.tile([128, 2 * R], f32, tag="qkv")
                psk = kvq[:, 0:R]
                psv = kvq[:, R:2 * R]
                for c in range(ET):
                    nc.tensor.matmul(psk, win_ap(c, 1, h), x2t_sb[c][:],
                                     start=(c == 0), stop=(c == ET - 1),
                                     skip_group_check=True)
                kvsb = smqp.tile([128, 2 * R], bf16, tag="kvloc")
                nc.vector.tensor_scalar_add(kvsb[:, 0:R], psk,
                                            bp_sb[:, BP_BK + h:BP_BK + h + 1])
                # V tile-packed [128, 4*128] (no bias: folded into out bias)
                for mt in range(ET):
                    for c in range(ET):
                        nc.tensor.matmul(
                            psv[:, mt * HD:(mt + 1) * HD],
                            x2t_sb[c][:, mt * 128:(mt + 1) * 128],
                            win_ap(c, 2, h),
                            start=(c == 0), stop=(c == ET - 1),
                            skip_group_check=True)
                nc.vector.tensor_copy(kvsb[:, R:2 * R], psv)
                # bounce + AllGather: rank block = [KT(128) ; Vpack(128)]
                kvi = drp.tile([256, R], bf16, tag=f"kvi{h}")
                kvo = nc.dram_tensor(f"kvo{h}", [NC_ * 256, R], bf16,
                                     kind="Internal", addr_space="Shared")
                nc.scalar.dma_start(
                    kvi[:, :].rearrange("(t p) j -> p t j", t=2),
                    kvsb[:].rearrange("p (t j) -> p t j", t=2))
                if sim1:
                    nc.scalar.dma_start(kvo[0:256, :], kvi[:])
                else:
                    nc.gpsimd.collective_compute(
                        "AllGather", ALU.bypass, replica_groups=RG,
                        ins=[kvi.opt()], outs=[kvo.ap()])
                kv_out[h] = kvo
                # Q^T [d, own nodes] (overlaps the AllGather); own ring slot
                # (the ring's WAR dep serializes it behind kt/vcopy reads)
                qq = psqkv.tile([128, 2 * R], f32, tag="qkv")
                psq = qq[:, 0:R]
                for c in range(ET):
                    nc.tensor.matmul(psq, win_ap(c, 0, h), x2t_sb[c][:],
                                     start=(c == 0), stop=(c == ET - 1),
                                     skip_group_check=True)
                qt = kvp.tile([128, R], bf16, tag="qt")
                nc.vector.tensor_scalar_add(qt[:], psq,
                                            bp_sb[:, BP_BQ + h:BP_BQ + h + 1])
                qt_sb[h] = qt
                # gathered K/V loads issued here (sync queue) so earlier
                # heads' loads aren't stuck behind later heads' bounce writes
                kv3 = kvo[:, :].rearrange("(r t p) j -> t p r j", t=2, p=128)
                ktg = kvp.tile([128, NC_ * R], bf16, tag="ktg")
                nc.sync.dma_start(
                    ktg[:].rearrange("p (r j) -> p r j", r=NC_), kv3[0])
                vg = kvp.tile([128, NC_ * R], bf16, tag="vg")
                nc.sync.dma_start(
                    vg[:].rearrange("p (r j) -> p r j", r=NC_), kv3[1])
                kv_out[h] = (ktg, vg)

            # ---------------- attention per head ----------------
            # NOTE: start_tensor_calc clears has_written for the WHOLE PSUM
            # bank, so accumulation groups must not interleave within a bank
            # (a later group's start turns an earlier group's next accumulate
            # into an overwrite). u is therefore accumulated in the tail with
            # strictly sequential groups.
            zhold = []
            GRP = 2            # key chunks per score tile
            NG = KB // GRP     # 16 groups
            for h in range(HEADS):
                ktg, vg = kv_out[h]
                ps_ctx = psctx.tile([128, R], f32, tag="ctx")
                acc = smqp.tile([128, GRP * R], bf16, tag="acc")
                # software-pipelined: PV of group g is emitted alongside the
                # score matmuls of group g+2, so the in-order PE queue never
                # blocks on an exp that hasn't finished.
                SKEW = 2
                pts = {}
                for g in range(NG + SKEW):
                    if g < NG:
                        psc = psbig.tile([128, GRP * R], f32, tag="sc")
                        for i in range(GRP):
                            kc = g * GRP + i
                            nc.tensor.matmul(psc[:, i * R:(i + 1) * R],
                                             ktg[:, kc * 128:(kc + 1) * 128],
                                             qt_sb[h][:], start=True, stop=True,
                                             skip_group_check=True)
                        pt = ptp.tile([128, GRP * R], bf16, tag="pt")
                        nc.scalar.activation(pt[:], psc[:], AF.Exp,
                                             scale=inv_sqrt_hd)
                        pts[g] = pt
                        if g == 0:
                            nc.vector.tensor_copy(acc[:], pt[:])
                        else:
                            nc.vector.tensor_add(acc[:], acc[:], pt[:])
                    gp = g - SKEW
                    if gp >= 0:
                        pt = pts.pop(gp)
                        for i in range(GRP):
                            kc = gp * GRP + i
                            nc.tensor.matmul(ps_ctx[:],
                                             vg[:, kc * 128:(kc + 1) * 128],
                                             pt[:, i * R:(i + 1) * R],
                                             start=(gp == 0 and i == 0),
                                             stop=(gp == NG - 1 and i == GRP - 1),
                                             skip_group_check=True)
                # denominator: partition-reduce acc via ones matmul
                ps_den = pssm.tile([1, R], f32, tag="sm")
                for i in range(GRP):
                    nc.tensor.matmul(ps_den[:], ones_b[:],
                                     acc[:, i * R:(i + 1) * R],
                                     start=(i == 0), stop=(i == GRP - 1),
                                     skip_group_check=True)
                rden = smp.tile([1, R], f32, tag="rden")
                nc.vector.reciprocal(rden[:], ps_den[:])
                rbc = smp.tile([128, R], f32, tag="rbc")
                nc.gpsimd.partition_broadcast(rbc[:], rden[:])
                ctxn = smp.tile([128, R], f32, tag="ctxn")
                nc.vector.tensor_mul(ctxn[:], ps_ctx[:], rbc[:])
                zf = smqp.tile([128, 1], f32, tag=f"z{h}")
                nc.vector.tensor_reduce(zf[:], ctxn[:], axis=AX.X, op=ALU.add)
                zhold.append(zf)

            # ---------------- out_proj + mean + fc (partial) ----------------
            psu = psqkv.tile([128, ET], f32, tag="qkv")
            u_sb = []
            for et in range(ET):
                for h in range(HEADS):
                    wo_c = bp_sb[:, BP_WO + h * G2 + et * 128:
                                 BP_WO + h * G2 + (et + 1) * 128]
                    nc.tensor.matmul(psu[:, et:et + 1], wo_c, zhold[h][:],
                                     start=(h == 0), stop=(h == HEADS - 1),
                                     skip_group_check=True)
                ut = smp.tile([128, 1], f32, tag=f"u{et}")
                nc.vector.tensor_scalar_mul(ut[:], psu[:, et:et + 1],
                                            1.0 / float(N))
                nc.vector.tensor_add(ut[:], ut[:],
                                     bp_sb[:, BP_BO + et:BP_BO + et + 1])
                u_sb.append(ut)
            ps_fc = pssm.tile([1, 2], f32, tag="sm")
            for c in range(ET):
                nc.tensor.matmul(ps_fc[:], u_sb[c][:],
                                 bp_sb[:, BP_FCW + c * 2:BP_FCW + (c + 1) * 2],
                                 start=(c == 0), stop=(c == ET - 1))
            ores = smp.tile([1, 2], f32, tag="ores")
            nc.vector.tensor_add(ores[:], ps_fc[:],
                                 bp_sb[0:1, BP_FCB:BP_FCB + 2])
            nc.sync.dma_start(out_d[:, :], ores[:])

    nc.compile()
    return nc


def _pack_tiles(a, p=128):
    """[n*p, c] row-major -> [p, n*c] tile-packed (partition = row % p)."""
    n = a.shape[0] // p
    return np.ascontiguousarray(
        a.reshape(n, p, a.shape[1]).transpose(1, 0, 2).reshape(p, -1))


def kernel(**inputs):
    from concourse.bass_utils import run_bass_kernel_spmd

    if "nc" not in _cache:
        _cache["nc"] = _build()
    nc = _cache["nc"]

    bf = ml_dtypes.bfloat16
    adj = np.ascontiguousarray(inputs["adj_matrix"], dtype=np.float32)
    x = np.ascontiguousarray(inputs["node_features"], dtype=np.float32)
    w1 = np.asarray(inputs["W1"], np.float32)
    b1 = np.asarray(inputs["b1"], np.float32)
    w2 = np.asarray(inputs["W2"], np.float32)
    b2 = np.asarray(inputs["b2"], np.float32)
    win = np.asarray(inputs["in_proj_w"], np.float32)
    bin_ = np.asarray(inputs["in_proj_b"], np.float32)
    wo = np.asarray(inputs["out_proj_w"], np.float32)
    bo = np.asarray(inputs["out_proj_b"], np.float32)
    fcw = np.asarray(inputs["fc_w"], np.float32)
    fcb = np.asarray(inputs["fc_b"], np.float32)

    # ---- constant-parameter folding (host, exact math on fp32 weights) ----
    # x2 is only consumed by the QKV projections, so fold b2 through them.
    bq_eff = b2 @ win[:, 0:G2] + bin_[0:G2]
    bk_eff = b2 @ win[:, G2:2 * G2] + bin_[G2:2 * G2]
    bv_eff = b2 @ win[:, 2 * G2:3 * G2] + bin_[2 * G2:3 * G2]
    # V bias passes through softmax untouched (weights sum to 1), so it adds
    # bv_eff @ wo to every row of attn_out; fold into the mean+out_proj bias.
    bo_eff8 = (bo + bv_eff @ wo) / NC_

    wp = np.concatenate([
        w1, w2, _pack_tiles(win)], axis=1).astype(bf)
    bp = np.zeros((128, BP_COLS), np.float32)
    bp[:, BP_BQ:BP_BQ + 4] = bq_eff.reshape(4, 128).T
    bp[:, BP_BK:BP_BK + 4] = bk_eff.reshape(4, 128).T
    bp[:, BP_B1:BP_B1 + G1] = np.broadcast_to(b1, (128, G1))
    bp[:, BP_BO:BP_BO + 4] = bo_eff8.reshape(4, 128).T
    bp[:, BP_WO:BP_WO + ET * G2] = _pack_tiles(wo)
    bp[:, BP_FCW:BP_FCW + 8] = _pack_tiles(fcw)
    bp[0, BP_FCB:BP_FCB + 2] = fcb / NC_

    xp = _pack_tiles(x).astype(bf)
    reps = {"xp": xp, "wp": wp, "bp": bp}

    in_maps = []
    idx = np.arange(R)
    for r in range(NC_):
        cols = np.ascontiguousarray(adj[:, r * R:(r + 1) * R])
        cols[r * R + idx, idx] += 1.0   # A + I, this core's diagonal block
        in_maps.append({"adjp": _pack_tiles(cols).astype(bf), **reps})

    res = run_bass_kernel_spmd(nc, in_maps, core_ids=list(range(NC_)))
    out = np.zeros(2, dtype=np.float64)
    for r in range(NC_):
        out += res.results[r]["outp"].reshape(2).astype(np.float64)
    return out.astype(np.float32)
